# revision 1
# baseline (speedup 1.0000x reference)
"""Multi-head self-attention (B=4, T=2048, E=1024, H=16) on 8 trn2 NeuronCores.

Sharding: core (b, h) = batch b, token-half h. Each core computes K/V for the
full sequence (duplicated within the batch pair), Q for its own 8 query blocks
of 128 tokens, causal attention for those blocks, then the output projection
and LayerNorm for its own tokens. Causal balance: query blocks are paired
(j, 15-j) so both cores process blocks with padded key-lengths 2,4,...,16;
host-supplied mask tiles encode the true causal structure, keeping the
compiled program identical across cores (SPMD).

All matmuls run in bf16 with fp32 PSUM accumulation (validated ~2e-3
scale-relative error vs the fp32 reference).
"""
import json
import numpy as np
import ml_dtypes
from contextlib import ExitStack

import concourse.bass as bass
import concourse.bass_utils as _bass_utils
import concourse.tile as tile
from concourse import mybir
from concourse.bass_utils import run_bass_kernel_spmd

# ----------------------------------------------------------------------------
# Toolchain workarounds for this container's walrus build (see birfix notes):
# 1. EVENT_SEMAPHORE_RANGE_CLEAR InstISA is rejected ("ISA wrong length").
# 2. Engine instructions only carry one semaphore-wait slot; extra waits are
#    peeled onto NoOp carriers on the same engine (order-preserving).
# ----------------------------------------------------------------------------


def _patched_clear_and_free_semaphores(self, sems):
    if not sems:
        return
    sem_nums = [s.num if hasattr(s, "num") else s for s in sems]
    self._state.prepend_free_semaphores(sem_nums)
    for poison_set in self._tile_sem_poison_stack:
        poison_set.update(sem_nums)


def _fix_bir_waits(bir_json: bytes) -> bytes:
    bir = json.loads(bir_json)
    ctr = 0
    changed = False
    for func in bir.get("functions", []):
        for blk in func.get("blocks", []):
            out = []
            for inst in blk.get("instructions", []):
                si = inst.get("sync_info") or {}
                waits = si.get("on_wait") or []
                if len(waits) > 1:
                    for w in waits[:-1]:
                        ctr += 1
                        out.append(
                            {
                                "debug": inst.get("debug"),
                                "engine": inst.get("engine", "SP"),
                                "ins": [],
                                "name": f"IWF-{ctr}",
                                "opcode": "NoOp",
                                "outs": [],
                                "sync_info": {"on_wait": [w]},
                            }
                        )
                    si = dict(si)
                    si["on_wait"] = waits[-1:]
                    inst = dict(inst)
                    inst["sync_info"] = si
                    changed = True
                out.append(inst)
            blk["instructions"] = out
    return json.dumps(bir).encode() if changed else bir_json


_orig_compile_bir_kernel = _bass_utils.compile_bir_kernel


def _patched_compile_bir_kernel(bir_json, tmpdir, neff_name="file.neff"):
    if isinstance(bir_json, str):
        bir_json = bir_json.encode()
    return _orig_compile_bir_kernel(_fix_bir_waits(bir_json), tmpdir, neff_name)


def _install_patches():
    if getattr(bass.Bass, "_mhsa_patched", False):
        return
    bass.Bass.clear_and_free_semaphores = _patched_clear_and_free_semaphores
    bass.Bass._mhsa_patched = True
    _bass_utils.compile_bir_kernel = _patched_compile_bir_kernel
    try:
        import concourse.bass2jax as _b2j

        _b2j.compile_bir_kernel = _patched_compile_bir_kernel
    except ImportError:
        pass


_install_patches()

# ----------------------------------------------------------------------------
# Problem constants (hardcoded per spec)
# ----------------------------------------------------------------------------
B, T, E, H = 4, 2048, 1024, 16
HD = E // H  # 64
P = 128
NB = T // P  # 16 query/key blocks
NQ = 8  # query blocks per core
EC = E // P  # 8 e-chunks
SCALE = 1.0 / float(np.sqrt(T))
EPS = 1e-6
BF = mybir.dt.bfloat16
F32 = mybir.dt.float32
NPBF = ml_dtypes.bfloat16

# query-block assignment: pairs (j, 15-j); core h=0 takes even-j pairs' low
# and high ends so both cores see padded lengths L_k = 2(k+1)
BLOCKS_A = [0, 2, 4, 6, 9, 11, 13, 15]  # true lengths 1,3,5,7,10,12,14,16
BLOCKS_B = [1, 3, 5, 7, 8, 10, 12, 14]  # true lengths 2,4,6,8,9,11,13,15
PAD_L = [2 * (k + 1) for k in range(NQ)]  # 2,4,...,16

_nc_cache = {}


def _build_nc():
    if "nc" in _nc_cache:
        return _nc_cache["nc"]
    nc = bass.Bass(num_devices=8)

    # inputs (per-core)
    xT_d = nc.dram_tensor("xT", [E, T], BF, kind="ExternalInput")
    xTq_d = nc.dram_tensor("xTq", [E, NQ * P], BF, kind="ExternalInput")
    WqT_d = nc.dram_tensor("WqT", [E, E], BF, kind="ExternalInput")
    WkT_d = nc.dram_tensor("WkT", [E, E], BF, kind="ExternalInput")
    WvT_d = nc.dram_tensor("WvT", [E, E], BF, kind="ExternalInput")
    WpT_d = nc.dram_tensor("WpT", [E, E], BF, kind="ExternalInput")
    bqT_d = nc.dram_tensor("bqT", [P, EC], F32, kind="ExternalInput")
    bkT_d = nc.dram_tensor("bkT", [P, EC], F32, kind="ExternalInput")
    bv_d = nc.dram_tensor("bv_bc", [P, E], BF, kind="ExternalInput")
    bp_d = nc.dram_tensor("bp_bc", [P, E], BF, kind="ExternalInput")
    gm_d = nc.dram_tensor("gamma_bc", [P, E], BF, kind="ExternalInput")
    bt_d = nc.dram_tensor("beta_bc", [P, E], BF, kind="ExternalInput")
    m1_d = nc.dram_tensor("m1", [P, NQ, P], BF, kind="ExternalInput")
    m2_d = nc.dram_tensor("m2", [P, NQ, P], BF, kind="ExternalInput")
    y_d = nc.dram_tensor("y", [NQ, P, E], F32, kind="ExternalOutput")

    with tile.TileContext(nc) as tc:
        with ExitStack() as ctx:
            consts = ctx.enter_context(tc.tile_pool(name="consts", bufs=1))
            big = ctx.enter_context(tc.tile_pool(name="big", bufs=1))
            wpool = ctx.enter_context(tc.tile_pool(name="wpool", bufs=1))
            work = ctx.enter_context(tc.tile_pool(name="work", bufs=2))
            # QKV-phase PSUM pool; closed before attention so its banks are
            # reused by the attention pool (8-bank budget)
            _psA_cm = tc.tile_pool(name="psA", bufs=1, space="PSUM")
            ps = _psA_cm.__enter__()

            def load_w(dram, name):
                # two half-tiles in a 3-slot rotation: the next projection's
                # first half streams in while the previous one's second half
                # is still being consumed
                halves = []
                for hf in range(2):
                    w = wpool.tile(
                        [P, EC, E // 2], BF, tag="wh", bufs=3, name=f"{name}{hf}"
                    )
                    for c in range(EC):
                        nc.sync.dma_start(
                            w[:, c, :],
                            dram.rearrange("(c p) f -> p c f", p=P)[
                                :, c, hf * 512 : (hf + 1) * 512
                            ],
                        )
                    halves.append(w)
                return halves

            # PE-critical loads first: Wk then xT, so the K matmuls can
            # start as soon as possible
            Wk = load_w(WkT_d, "Wk")
            xT = big.tile([P, EC, T], BF)
            for c in range(EC):
                nc.sync.dma_start(
                    xT[:, c, :], xT_d.rearrange("(c p) t -> p c t", p=P)[:, c, :]
                )
            bkT = consts.tile([P, EC], F32)
            nc.sync.dma_start(bkT[:], bkT_d[:, :])
            bv_bc = consts.tile([P, E], BF)
            nc.sync.dma_start(bv_bc[:], bv_d[:, :])
            xTq = big.tile([P, EC, NQ * P], BF)
            for c in range(EC):
                nc.sync.dma_start(
                    xTq[:, c, :], xTq_d.rearrange("(c p) t -> p c t", p=P)[:, c, :]
                )
            bqT = consts.tile([P, EC], F32)
            nc.sync.dma_start(bqT[:], bqT_d[:, :])
            bp_bc = consts.tile([P, E], BF)
            nc.sync.dma_start(bp_bc[:], bp_d[:, :])
            gamma_bc = consts.tile([P, E], BF)
            nc.sync.dma_start(gamma_bc[:], gm_d[:, :])
            beta_bc = consts.tile([P, E], BF)
            nc.sync.dma_start(beta_bc[:], bt_d[:, :])
            m1 = consts.tile([P, NQ, P], BF)
            nc.sync.dma_start(m1[:], m1_d[:, :, :])
            m2 = consts.tile([P, NQ, P], BF)
            nc.sync.dma_start(m2[:], m2_d[:, :, :])
            ones64 = consts.tile([P, 64], F32)
            nc.vector.memset(ones64[:], 1.0)

            # persistent intermediates
            KT = big.tile([P, EC, T], BF)  # K^T  [f, t]
            QT = big.tile([P, EC, NQ * P], BF)  # Q^T  [f, t_own]
            Vx = big.tile([P, NB, H, HD + 1], BF)  # V ext [t, h, d|1]
            zT = big.tile([P, EC, NQ * P], BF)  # z^T  [e, t_own]
            nc.vector.memset(Vx[:, :, :, HD : HD + 1], 1.0)

            # ---------------- K^T = Wk^T.T-chunks x xT + bk ----------------
            for fb in range(EC):
                for ts_ in range(T // 512):
                    pk = ps.tile([P, 512], F32, tag="mm512", bufs=4, name="pk")
                    for c in range(EC):
                        nc.tensor.matmul(
                            pk[:],
                            Wk[fb // 4][:, c, (fb % 4) * P : (fb % 4 + 1) * P],
                            xT[:, c, ts_ * 512 : (ts_ + 1) * 512],
                            start=(c == 0),
                            stop=(c == EC - 1),
                        )
                    nc.vector.tensor_scalar(
                        out=KT[:, fb, ts_ * 512 : (ts_ + 1) * 512],
                        in0=pk[:],
                        scalar1=bkT[:, fb : fb + 1],
                        scalar2=None,
                        op0=mybir.AluOpType.add,
                    )

            # ---------------- V = xT-chunks x Wv^T + bv (t-major, ext) -----
            Wv = load_w(WvT_d, "Wv")
            for tb in range(NB):
                for fs in range(E // 512):
                    pv = ps.tile([P, 512], F32, tag="mm512", bufs=4, name="pv")
                    for c in range(EC):
                        nc.tensor.matmul(
                            pv[:],
                            xT[:, c, tb * P : (tb + 1) * P],
                            Wv[fs][:, c, :],
                            start=(c == 0),
                            stop=(c == EC - 1),
                        )
                    nc.vector.tensor_tensor(
                        out=Vx[:, tb, fs * 8 : (fs + 1) * 8, 0:HD],
                        in0=pv[:, :].rearrange("p (h d) -> p h d", d=HD),
                        in1=bv_bc[:, fs * 512 : (fs + 1) * 512].rearrange(
                            "p (h d) -> p h d", d=HD
                        ),
                        op=mybir.AluOpType.add,
                    )

            # ---------------- Q^T = Wq^T-chunks x xTq + bq -----------------
            Wq = load_w(WqT_d, "Wq")
            for fb in range(EC):
                for ts_ in range(NQ * P // 512):
                    pq = ps.tile([P, 512], F32, tag="mm512", bufs=4, name="pq")
                    for c in range(EC):
                        nc.tensor.matmul(
                            pq[:],
                            Wq[fb // 4][:, c, (fb % 4) * P : (fb % 4 + 1) * P],
                            xTq[:, c, ts_ * 512 : (ts_ + 1) * 512],
                            start=(c == 0),
                            stop=(c == EC - 1),
                        )
                    nc.vector.tensor_scalar(
                        out=QT[:, fb, ts_ * 512 : (ts_ + 1) * 512],
                        in0=pq[:],
                        scalar1=bqT[:, fb : fb + 1],
                        scalar2=None,
                        op0=mybir.AluOpType.add,
                    )

            # ---------------- attention ----------------
            # swap PSUM pools: QKV pool's banks get reused for attention
            _psA_cm.__exit__(None, None, None)
            _psB_cm = tc.tile_pool(name="psB", bufs=1, space="PSUM")
            ps = _psB_cm.__enter__()
            def emit_sgroup(pr, qs, g0, gw):
                # one 2-bank psum: cols 0:512 even head, 512:1024 odd head
                pS = ps.tile([P, 1024], F32, tag="S", bufs=3, name="pS")
                for jj in range(gw):
                    js = slice((g0 + jj) * P, (g0 + jj + 1) * P)
                    nc.tensor.matmul(
                        pS[:, jj * P : (jj + 1) * P],
                        KT[0:64, pr, js],
                        QT[0:64, pr, qs],
                        start=True,
                        stop=True,
                        tile_position=(0, 0),
                    )
                    nc.tensor.matmul(
                        pS[:, 512 + jj * P : 512 + (jj + 1) * P],
                        KT[64:128, pr, js],
                        QT[64:128, pr, qs],
                        start=True,
                        stop=True,
                        tile_position=(64, 0),
                    )
                return pS

            def emit_division_pair(h_e, pOe, h_o, pOo, qs):
                # per head: copy the sums row to SBUF (DVE), broadcast across
                # 64 partitions with a K=1 matmul, reciprocal, multiply, and
                # scatter into z^T. Both sm copies go first so the PE
                # broadcasts never sit behind other DVE work.
                sms = []
                for pO in (pOe, pOo):
                    sm = work.tile([P, P], F32, tag="sm", bufs=2, name="sm")
                    nc.vector.tensor_copy(sm[64:65, :], pO[64:65, :])
                    sms.append(sm)
                # broadcast into the unused rows 64:128 of the pO bank itself
                for sm, pO in zip(sms, (pOe, pOo)):
                    nc.tensor.matmul(
                        pO[64:128, :], ones64[64:65, :], sm[64:65, :],
                        start=True, stop=True,
                    )
                for h, pO in ((h_e, pOe), (h_o, pOo)):
                    Rs = work.tile([64, P], F32, tag="Rs", bufs=2, name="Rs")
                    nc.vector.reciprocal(Rs[:], pO[64:128, :])
                    zh = work.tile([64, P], BF, tag="zh", bufs=4, name="zh")
                    nc.vector.tensor_tensor(
                        out=zh[:], in0=pO[0:HD, :], in1=Rs[:],
                        op=mybir.AluOpType.mult,
                    )
                    nc.sync.dma_start(
                        zT[(h % 2) * 64 : (h % 2) * 64 + 64, h // 2, qs], zh[:]
                    )

            # flat list of (unit_idx, k_idx, pr, g0, gw); one unit = head pair
            units = []
            flat = []
            for k_idx in range(NQ):
                L = PAD_L[k_idx]
                for pr in range(H // 2):
                    u = len(units)
                    units.append((k_idx, pr, L))
                    for g0 in range(0, L, 4):
                        flat.append((u, g0, min(4, L - g0)))

            pO_cur = None
            pending_div = None
            prev_S = None

            def sgroup_for(idx):
                u, g0, gw = flat[idx]
                k_idx, pr, L = units[u]
                return emit_sgroup(
                    pr, slice(k_idx * P, (k_idx + 1) * P), g0, gw
                )

            prev_S = sgroup_for(0)
            for i, (u, g0, gw) in enumerate(flat):
                k_idx, pr, L = units[u]
                qs = slice(k_idx * P, (k_idx + 1) * P)
                h_e, h_o = 2 * pr, 2 * pr + 1
                if g0 == 0:
                    pO_cur = (
                        ps.tile([P, P], F32, tag="Oe", bufs=1, name="pOe"),
                        ps.tile([P, P], F32, tag="Oo", bufs=1, name="pOo"),
                    )
                pOe, pOo = pO_cur
                pS = prev_S
                w = gw * P
                eS = work.tile([P, 1024], BF, tag="eS", bufs=3, name="eS")
                nc.scalar.activation(
                    eS[:, :].rearrange("p (u q) -> p u q", u=2)[:, :, 0:w],
                    pS[:, :].rearrange("p (u q) -> p u q", u=2)[:, :, 0:w],
                    mybir.ActivationFunctionType.Exp,
                    scale=SCALE,
                )
                if i + 1 < len(flat):
                    # next score group (possibly of the next head pair)
                    # issues on PE while ACT runs this group's exp
                    prev_S = sgroup_for(i + 1)
                if pending_div is not None and g0 == 0:
                    pending_div()
                    pending_div = None
                for jj in range(gw):
                    j = g0 + jj
                    cs = slice(jj * P, (jj + 1) * P)
                    if j >= L - 2:
                        m = m1 if j == L - 2 else m2
                        nc.vector.tensor_tensor(
                            out=eS[:, :].rearrange("p (u q) -> p u q", u=2)[
                                :, :, cs
                            ],
                            in0=eS[:, :].rearrange("p (u q) -> p u q", u=2)[
                                :, :, cs
                            ],
                            in1=m[:, k_idx : k_idx + 1, :].to_broadcast(
                                (P, 2, P)
                            ),
                            op=mybir.AluOpType.mult,
                        )
                    nc.tensor.matmul(
                        pOe[0 : HD + 1, :],
                        Vx[:, j, h_e, :],
                        eS[:, cs],
                        start=(j == 0),
                        stop=(j == L - 1),
                    )
                    nc.tensor.matmul(
                        pOo[0 : HD + 1, :],
                        Vx[:, j, h_o, :],
                        eS[:, 512 + jj * P : 512 + (jj + 1) * P],
                        start=(j == 0),
                        stop=(j == L - 1),
                    )
                if g0 + gw == L:

                    def _div(h_e=h_e, h_o=h_o, pOe=pOe, pOo=pOo, qs=qs):
                        emit_division_pair(h_e, pOe, h_o, pOo, qs)

                    pending_div = _div
            if pending_div is not None:
                pending_div()
                pending_div = None

            # residual: z^T += xTq
            for c in range(EC):
                nc.vector.tensor_tensor(
                    out=zT[:, c, :], in0=zT[:, c, :], in1=xTq[:, c, :],
                    op=mybir.AluOpType.add,
                )

            # ---------------- projection + layernorm ----------------
            _psB_cm.__exit__(None, None, None)
            _psC_cm = tc.tile_pool(name="psC", bufs=1, space="PSUM")
            ps = _psC_cm.__enter__()
            Wp = load_w(WpT_d, "Wp")
            inv_e = 1.0 / float(E)
            for tb in range(NQ):
                y_sb = work.tile([P, E], F32, tag="ysb", bufs=2, name="y_sb")
                for fs in range(E // 512):
                    py = ps.tile([P, 512], F32, tag="mm512", bufs=4, name="py")
                    for c in range(EC):
                        nc.tensor.matmul(
                            py[:],
                            zT[:, c, tb * P : (tb + 1) * P],
                            Wp[fs][:, c, :],
                            start=(c == 0),
                            stop=(c == EC - 1),
                        )
                    nc.vector.tensor_tensor(
                        out=y_sb[:, fs * 512 : (fs + 1) * 512],
                        in0=py[:],
                        in1=bp_bc[:, fs * 512 : (fs + 1) * 512],
                        op=mybir.AluOpType.add,
                    )
                mean = work.tile([P, 1], F32, tag="stat", bufs=8, name="mean")
                nc.vector.reduce_sum(mean[:], y_sb[:], axis=mybir.AxisListType.X)
                nc.vector.tensor_scalar_mul(mean[:], mean[:], -inv_e)
                y_c = work.tile([P, E], F32, tag="yc", bufs=2, name="y_c")
                nc.scalar.activation(
                    y_c[:], y_sb[:], mybir.ActivationFunctionType.Identity,
                    bias=mean[:, 0:1],
                )
                var = work.tile([P, 1], F32, tag="stat", bufs=8, name="var")
                nc.scalar.activation(
                    y_sb[:], y_c[:], mybir.ActivationFunctionType.Square,
                    accum_out=var[:],
                )
                rstd = work.tile([P, 1], F32, tag="stat", bufs=8, name="rstd")
                nc.vector.tensor_scalar(
                    out=rstd[:], in0=var[:], scalar1=inv_e, scalar2=float(EPS),
                    op0=mybir.AluOpType.mult, op1=mybir.AluOpType.add,
                )
                nc.scalar.activation(
                    rstd[:], rstd[:], mybir.ActivationFunctionType.Sqrt
                )
                nc.vector.reciprocal(rstd[:], rstd[:])
                nc.scalar.activation(
                    y_sb[:], y_c[:], mybir.ActivationFunctionType.Identity,
                    scale=rstd[:, 0:1],
                )
                nc.vector.tensor_tensor(
                    out=y_c[:], in0=y_sb[:], in1=gamma_bc[:],
                    op=mybir.AluOpType.mult,
                )
                nc.vector.tensor_tensor(
                    out=y_c[:], in0=y_c[:], in1=beta_bc[:],
                    op=mybir.AluOpType.add,
                )
                nc.sync.dma_start(y_d[tb, :, :], y_c[:])

            _psC_cm.__exit__(None, None, None)

    _nc_cache["nc"] = nc
    return nc


def _make_masks(blocks):
    m1 = np.zeros((NQ, P, P), np.float32)
    m2 = np.zeros((NQ, P, P), np.float32)
    tril_t = (np.arange(P)[:, None] <= np.arange(P)[None, :]).astype(np.float32)
    for k in range(NQ):
        l_true = blocks[k] + 1
        L = PAD_L[k]
        if l_true == L:
            m1[k] = 1.0
            m2[k] = tril_t
        else:
            assert l_true == L - 1
            m1[k] = tril_t
            m2[k] = 0.0
    # device layout [P(k-local), NQ, P(q-local)]
    return (
        np.ascontiguousarray(m1.transpose(1, 0, 2)).astype(NPBF),
        np.ascontiguousarray(m2.transpose(1, 0, 2)).astype(NPBF),
    )


def kernel(x, Wq, bq, Wk, bk, Wv, bv, Wp, bp, gamma, beta):
    x = np.asarray(x, np.float32)
    nc = _build_nc()

    WqT = np.ascontiguousarray(np.asarray(Wq, np.float32).T).astype(NPBF)
    WkT = np.ascontiguousarray(np.asarray(Wk, np.float32).T).astype(NPBF)
    WvT = np.ascontiguousarray(np.asarray(Wv, np.float32).T).astype(NPBF)
    WpT = np.ascontiguousarray(np.asarray(Wp, np.float32).T).astype(NPBF)
    bqT = np.ascontiguousarray(np.asarray(bq, np.float32).reshape(EC, P).T)
    bkT = np.ascontiguousarray(np.asarray(bk, np.float32).reshape(EC, P).T)
    bv_bc = np.ascontiguousarray(
        np.broadcast_to(np.asarray(bv, np.float32), (P, E))
    ).astype(NPBF)
    bp_bc = np.ascontiguousarray(
        np.broadcast_to(np.asarray(bp, np.float32), (P, E))
    ).astype(NPBF)
    gamma_bc = np.ascontiguousarray(
        np.broadcast_to(np.asarray(gamma, np.float32), (P, E))
    ).astype(NPBF)
    beta_bc = np.ascontiguousarray(
        np.broadcast_to(np.asarray(beta, np.float32), (P, E))
    ).astype(NPBF)
    masks = {0: _make_masks(BLOCKS_A), 1: _make_masks(BLOCKS_B)}

    in_maps = []
    for core in range(8):
        b, h = core // 2, core % 2
        blocks = BLOCKS_A if h == 0 else BLOCKS_B
        own = np.concatenate([np.arange(blk * P, (blk + 1) * P) for blk in blocks])
        xb = x[b]  # (T, E)
        xT = np.ascontiguousarray(xb.T).astype(NPBF)
        xTq = np.ascontiguousarray(xb[own].T).astype(NPBF)
        m1c, m2c = masks[h]
        in_maps.append(
            {
                "xT": xT,
                "xTq": xTq,
                "WqT": WqT,
                "WkT": WkT,
                "WvT": WvT,
                "WpT": WpT,
                "bqT": bqT,
                "bkT": bkT,
                "bv_bc": bv_bc,
                "bp_bc": bp_bc,
                "gamma_bc": gamma_bc,
                "beta_bc": beta_bc,
                "m1": m1c,
                "m2": m2c,
            }
        )

    import os

    trace = bool(int(os.environ.get("MHSA_TRACE", "0")))
    res = run_bass_kernel_spmd(
        nc, in_maps, core_ids=list(range(8)), trace=trace,
        trace_cores=list(range(8)) if trace else None,
    )
    if trace and res.exec_time_ns is not None:
        print(f"HW exec time: {res.exec_time_ns} ns")
        if res.mean_exec_time_ns is not None:
            print(f"HW exec mean across cores: {res.mean_exec_time_ns:.0f} ns")
        kernel.last_exec_time_ns = res.exec_time_ns
        kernel.last_trace = res.instructions_and_trace

    out = np.empty((B, T, E), np.float32)
    for core in range(8):
        b, h = core // 2, core % 2
        blocks = BLOCKS_A if h == 0 else BLOCKS_B
        y = res.results[core]["y"]  # (NQ, P, E)
        for k, blk in enumerate(blocks):
            out[b, blk * P : (blk + 1) * P, :] = y[k]
    return out



# revision 45
# speedup vs baseline: 1.4500x; 1.4500x over previous
"""Multi-head self-attention (B=4, T=2048, E=1024, H=16) on 8 trn2 NeuronCores.

Sharding: core (b, h) = batch b, token-half h. Each core computes K/V for the
full sequence (duplicated within the batch pair), Q for its own 8 query blocks
of 128 tokens, causal attention for those blocks, then the output projection
and LayerNorm for its own tokens. Causal balance: query blocks are paired
(j, 15-j) so both cores process blocks with padded key-lengths 2,4,...,16;
host-supplied mask tiles encode the true causal structure, keeping the
compiled program identical across cores (SPMD).

Perf structure (cost-model driven). The kernel is one fused pipeline whose
rate limiter is the softmax exp on the ACT engine (~150us of the ~190us
total), so everything else is arranged to hide under it:
- Q/K/V projections run as fp8e4 DoubleRow matmuls (2x128 contraction per
  instruction at 0.5 cycles/row, 4x fewer PE cycles than bf16) and are
  emitted just-in-time inside the attention loop: K/Q per-head-pair chunks
  during the first query block, V blocks prefetched one window ahead.
- att@V uses exp-scores as the stationary operand and extended V (ones
  column for the softmax denominator) as the moving one, producing [q, d]
  output at 65 moving columns per instruction; the denominator lands in a
  per-partition scalar, so the division is reciprocal + one multiply.
- z is transposed back to [e, t] with PE transposes into a bf16-bitcast
  view of a f32 PSUM tile; the residual add is fused into the single drain.
- The projection + LayerNorm for query block q runs inside window q+1 of
  the attention loop, split into two stages so no engine queue blocks on
  the LN dependency chain; causal masks run on the idle GPSIMD engine.
- Input DMAs are issued alternately from the SP and ACT queues in
  criticality order (x, Wk, Wq, xq, masks, Wv, ...).
"""
import itertools
import json
import numpy as np
import ml_dtypes
from contextlib import ExitStack

import concourse.bass as bass
import concourse.bass_utils as _bass_utils
import concourse.tile as tile
from concourse import mybir
from concourse.bass_utils import run_bass_kernel_spmd

# ----------------------------------------------------------------------------
# Toolchain workarounds for this container's walrus build (see birfix notes):
# 1. EVENT_SEMAPHORE_RANGE_CLEAR InstISA is rejected ("ISA wrong length").
# 2. Engine instructions only carry one semaphore-wait slot; extra waits are
#    peeled onto NoOp carriers on the same engine (order-preserving).
# ----------------------------------------------------------------------------


def _patched_clear_and_free_semaphores(self, sems):
    if not sems:
        return
    sem_nums = [s.num if hasattr(s, "num") else s for s in sems]
    self._state.prepend_free_semaphores(sem_nums)
    for poison_set in self._tile_sem_poison_stack:
        poison_set.update(sem_nums)


def _fix_bir_waits(bir_json: bytes) -> bytes:
    bir = json.loads(bir_json)
    ctr = 0
    changed = False
    for func in bir.get("functions", []):
        for blk in func.get("blocks", []):
            out = []
            for inst in blk.get("instructions", []):
                si = inst.get("sync_info") or {}
                waits = si.get("on_wait") or []
                if len(waits) > 1:
                    for w in waits[:-1]:
                        ctr += 1
                        out.append(
                            {
                                "debug": inst.get("debug"),
                                "engine": inst.get("engine", "SP"),
                                "ins": [],
                                "name": f"IWF-{ctr}",
                                "opcode": "NoOp",
                                "outs": [],
                                "sync_info": {"on_wait": [w]},
                            }
                        )
                    si = dict(si)
                    si["on_wait"] = waits[-1:]
                    inst = dict(inst)
                    inst["sync_info"] = si
                    changed = True
                out.append(inst)
            blk["instructions"] = out
    return json.dumps(bir).encode() if changed else bir_json


_orig_compile_bir_kernel = _bass_utils.compile_bir_kernel


def _patched_compile_bir_kernel(bir_json, tmpdir, neff_name="file.neff"):
    if isinstance(bir_json, str):
        bir_json = bir_json.encode()
    return _orig_compile_bir_kernel(_fix_bir_waits(bir_json), tmpdir, neff_name)


def _install_patches():
    if getattr(bass.Bass, "_mhsa_patched", False):
        return
    bass.Bass.clear_and_free_semaphores = _patched_clear_and_free_semaphores
    bass.Bass._mhsa_patched = True
    _bass_utils.compile_bir_kernel = _patched_compile_bir_kernel
    try:
        import concourse.bass2jax as _b2j

        _b2j.compile_bir_kernel = _patched_compile_bir_kernel
    except ImportError:
        pass


_install_patches()

# ----------------------------------------------------------------------------
# Problem constants (hardcoded per spec)
# ----------------------------------------------------------------------------
B, T, E, H = 4, 2048, 1024, 16
HD = E // H  # 64
P = 128
NB = T // P  # 16 query/key blocks
NQ = 8  # query blocks per core
EC = E // P  # 8 e-chunks
SCALE = 1.0 / float(np.sqrt(T))
EPS = 1e-6
BF = mybir.dt.bfloat16
F32 = mybir.dt.float32
F8 = mybir.dt.float8e4
NPBF = ml_dtypes.bfloat16
NPF8 = ml_dtypes.float8_e4m3
DR = mybir.MatmulPerfMode.DoubleRow

# query-block assignment: pairs (j, 15-j); core h=0 takes even-j pairs' low
# and high ends so both cores see padded lengths L_k = 2(k+1)
BLOCKS_A = [0, 2, 4, 6, 9, 11, 13, 15]  # true lengths 1,3,5,7,10,12,14,16
BLOCKS_B = [1, 3, 5, 7, 8, 10, 12, 14]  # true lengths 2,4,6,8,9,11,13,15
PAD_L = [2 * (k + 1) for k in range(NQ)]  # 2,4,...,16

_nc_cache = {}


def _build_nc():
    if "nc" in _nc_cache:
        return _nc_cache["nc"]
    nc = bass.Bass(num_devices=8)

    # inputs (per-core)
    xT8_d = nc.dram_tensor("xT8", [E, T], F8, kind="ExternalInput")
    xTq8_d = nc.dram_tensor("xTq8", [E, NQ * P], F8, kind="ExternalInput")
    xTq_d = nc.dram_tensor("xTq", [E, NQ * P], BF, kind="ExternalInput")
    Wq8_d = nc.dram_tensor("Wq8", [E, E], F8, kind="ExternalInput")
    Wk8_d = nc.dram_tensor("Wk8", [E, E], F8, kind="ExternalInput")
    Wv8_d = nc.dram_tensor("Wv8", [E, E], F8, kind="ExternalInput")
    WpT_d = nc.dram_tensor("WpT", [E, E], BF, kind="ExternalInput")
    bqT_d = nc.dram_tensor("bqT", [P, EC], F32, kind="ExternalInput")
    bkT_d = nc.dram_tensor("bkT", [P, EC], F32, kind="ExternalInput")
    bv_d = nc.dram_tensor("bv_bc", [P, E], BF, kind="ExternalInput")
    bp_d = nc.dram_tensor("bp_bc", [P, E], BF, kind="ExternalInput")
    gm_d = nc.dram_tensor("gamma_bc", [P, E], BF, kind="ExternalInput")
    bt_d = nc.dram_tensor("beta_bc", [P, E], BF, kind="ExternalInput")
    m1_d = nc.dram_tensor("m1", [P, NQ, P], BF, kind="ExternalInput")
    m2_d = nc.dram_tensor("m2", [P, NQ, P], BF, kind="ExternalInput")
    id_d = nc.dram_tensor("ident", [P, P], BF, kind="ExternalInput")
    # fp8 second-digit residuals: fix the V projection for the first two
    # token blocks, where causal attention averages few values and fp8
    # quantization error would dominate the output absmax
    xr8_d = nc.dram_tensor("xr8", [E, 2 * P], F8, kind="ExternalInput")
    Wvr8_d = nc.dram_tensor("Wvr8", [E, E], F8, kind="ExternalInput")
    y_d = nc.dram_tensor("y", [NQ, P, E], F32, kind="ExternalOutput")

    with tile.TileContext(nc) as tc:
        with ExitStack() as ctx:
            consts = ctx.enter_context(tc.tile_pool(name="consts", bufs=1))
            big = ctx.enter_context(tc.tile_pool(name="big", bufs=1))
            wpool = ctx.enter_context(tc.tile_pool(name="wpool", bufs=1))
            work = ctx.enter_context(tc.tile_pool(name="work", bufs=2))
            ps = ctx.enter_context(tc.tile_pool(name="ps", bufs=1, space="PSUM"))

            # Critical loads alternate between the SP and ACT issue queues
            # (halving serial issue cost); tail loads go SP-only so the ACT
            # sequencer is free once the exp stream begins. Transfers are
            # chunked ~256KB so the 16 DMA engines start the K-projection
            # inputs within ~10us.
            _dma_rr = itertools.cycle([nc.sync, nc.scalar])

            def dma(dst, src):
                _dma_rr.__next__().dma_start(dst, src)

            def dma_sp(dst, src):
                nc.sync.dma_start(dst, src)

            # --- critical loads ---
            bkT = consts.tile([P, EC], F32)
            dma(bkT[:], bkT_d[:, :])
            bqT = consts.tile([P, EC], F32)
            dma(bqT[:], bqT_d[:, :])
            bv_bc = consts.tile([P, E], BF)
            dma(bv_bc[:], bv_d[:, :])
            xT8 = big.tile([P, EC, T], F8)
            for c in range(EC):
                dma(xT8[:, c, :], xT8_d.rearrange("(c p) t -> p c t", p=P)[:, c, :])

            def load_w8(dram, name, issue=dma):
                w = wpool.tile([P, EC, E], F8, tag="w8", bufs=4, name=name)
                for c0 in range(0, EC, 2):
                    issue(
                        w[:, c0 : c0 + 2, :],
                        dram.rearrange("(c p) f -> p c f", p=P)[:, c0 : c0 + 2, :],
                    )
                return w

            Wk = load_w8(Wk8_d, "Wk")
            Wq = load_w8(Wq8_d, "Wq")
            xTq8 = big.tile([P, EC, NQ * P], F8)
            for c0 in range(0, EC, 4):
                dma(
                    xTq8[:, c0 : c0 + 4, :],
                    xTq8_d.rearrange("(c p) t -> p c t", p=P)[:, c0 : c0 + 4, :],
                )
            Wv = load_w8(Wv8_d, "Wv")
            Wvr = load_w8(Wvr8_d, "Wvr")
            xr8 = big.tile([P, EC, 2 * P], F8)
            dma(xr8[:], xr8_d.rearrange("(c p) t -> p c t", p=P)[:, :, :])
            m1 = consts.tile([P, NQ, P], BF)
            dma(m1[:], m1_d[:, :, :])
            m2 = consts.tile([P, NQ, P], BF)
            dma(m2[:], m2_d[:, :, :])
            ident = consts.tile([P, P], BF)
            dma(ident[:], id_d[:, :])
            # --- tail loads (SP queue only) ---
            xTq = big.tile([P, EC, NQ * P], BF)
            for c0 in range(0, EC, 2):
                dma_sp(
                    xTq[:, c0 : c0 + 2, :],
                    xTq_d.rearrange("(c p) t -> p c t", p=P)[:, c0 : c0 + 2, :],
                )
            Wp = []
            for hf in range(2):
                wph = wpool.tile(
                    [P, EC, E // 2], BF, tag="wp", bufs=2, name=f"Wp{hf}"
                )
                for c0 in range(0, EC, 2):
                    dma_sp(
                        wph[:, c0 : c0 + 2, :],
                        WpT_d.rearrange("(c p) f -> p c f", p=P)[
                            :, c0 : c0 + 2, hf * 512 : (hf + 1) * 512
                        ],
                    )
                Wp.append(wph)
            bp_bc = consts.tile([P, E], BF)
            dma_sp(bp_bc[:], bp_d[:, :])
            gamma_bc = consts.tile([P, E], BF)
            dma_sp(gamma_bc[:], gm_d[:, :])
            beta_bc = consts.tile([P, E], BF)
            dma_sp(beta_bc[:], bt_d[:, :])

            # persistent intermediates
            KT = big.tile([P, EC, T], BF)  # K^T  [f, t]
            QT = big.tile([P, EC, NQ * P], BF)  # Q^T  [f, t_own]
            Vx = big.tile([P, NB, H, HD + 1], BF)  # V ext [t, h, d|1]
            nc.vector.memset(Vx[:, :, :, HD : HD + 1], 1.0)

            inv_e = 1.0 / float(E)

            # ---- fp8 DoubleRow projection emitters ----
            def emit_kq_fb(fb):
                # K^T and Q^T rows for feature chunk fb (= head pair fb)
                for which, wt, rhs, ncols, dstT, bias in (
                    ("k", Wk, xT8, T, KT, bkT),
                    ("q", Wq, xTq8, NQ * P, QT, bqT),
                ):
                    for t2 in range(ncols // 512):
                        pk = ps.tile([P, 512], F32, tag="pz", bufs=4, name="pk")
                        for half in range(2):
                            ts_ = slice(
                                (t2 * 2 + half) * 256, (t2 * 2 + half) * 256 + 256
                            )
                            for cg in range(4):
                                nc.tensor.matmul(
                                    pk[:, half * 256 : half * 256 + 256],
                                    wt[:, 2 * cg : 2 * cg + 2, fb * P : (fb + 1) * P],
                                    rhs[:, 2 * cg : 2 * cg + 2, ts_],
                                    start=(cg == 0),
                                    stop=(cg == 3),
                                    perf_mode=DR,
                                )
                        nc.vector.tensor_scalar(
                            out=dstT[:, fb, t2 * 512 : (t2 + 1) * 512],
                            in0=pk[:],
                            scalar1=bias[:, fb : fb + 1],
                            scalar2=None,
                            op0=mybir.AluOpType.add,
                        )

            def emit_v_tb(tb):
                # V rows for token block tb; first two blocks get a
                # first-order fp8 residual correction (x_r@Wv + x@Wv_r)
                passes = [(xT8, Wv)]
                if tb < 2:
                    passes += [(xr8, Wv), (xT8, Wvr)]
                for f2 in range(2):
                    pv = ps.tile([P, 512], F32, tag="pz", bufs=4, name="pv")
                    for half in range(2):
                        fs = slice((f2 * 2 + half) * 256, (f2 * 2 + half) * 256 + 256)
                        for pi, (xs, ws) in enumerate(passes):
                            xcols = slice(tb * P, (tb + 1) * P)
                            for cg in range(4):
                                nc.tensor.matmul(
                                    pv[:, half * 256 : half * 256 + 256],
                                    xs[:, 2 * cg : 2 * cg + 2, xcols],
                                    ws[:, 2 * cg : 2 * cg + 2, fs],
                                    start=(pi == 0 and cg == 0),
                                    stop=(pi == len(passes) - 1 and cg == 3),
                                    perf_mode=DR,
                                )
                    nc.vector.tensor_tensor(
                        out=Vx[:, tb, f2 * 8 : (f2 + 1) * 8, 0:HD],
                        in0=pv[:, :].rearrange("p (h d) -> p h d", d=HD),
                        in1=bv_bc[:, f2 * 512 : (f2 + 1) * 512].rearrange(
                            "p (h d) -> p h d", d=HD
                        ),
                        op=mybir.AluOpType.add,
                    )

            # ---- attention score group ----
            def emit_sgroup(pr, qs, g0, gw):
                pS = ps.tile([P, 1024], F32, tag="S", bufs=2, name="pS")
                for jj in range(gw):
                    js = slice((g0 + jj) * P, (g0 + jj + 1) * P)
                    nc.tensor.matmul(
                        pS[:, jj * P : (jj + 1) * P],
                        KT[0:64, pr, js],
                        QT[0:64, pr, qs],
                        start=True,
                        stop=True,
                        tile_position=(0, 0),
                    )
                    nc.tensor.matmul(
                        pS[:, 512 + jj * P : 512 + (jj + 1) * P],
                        KT[64:128, pr, js],
                        QT[64:128, pr, qs],
                        start=True,
                        stop=True,
                        tile_position=(64, 0),
                    )
                return pS

            units = []
            flat = []
            for k_idx in range(NQ):
                L = PAD_L[k_idx]
                for pr in range(H // 2):
                    u = len(units)
                    units.append((k_idx, pr, L))
                    for g0 in range(0, L, 4):
                        flat.append((u, g0, min(4, L - g0)))

            def sgroup_for(idx):
                u, g0, gw = flat[idx]
                k_idx, pr, L = units[u]
                return emit_sgroup(pr, slice(k_idx * P, (k_idx + 1) * P), g0, gw)

            # ---- projection + LN pipeline (runs inside window qb+1),
            # split into small thunks so injected PE work never delays the
            # score pipeline by more than ~1us ----
            ln_state = {}

            def ln_tr(qb, half):
                # transpose z[q, e] -> [e, q] via PE into a bf16 view of a
                # f32 psum tile; drain fuses the residual add. half=0 covers
                # heads 0-7 (available right after pr3's division, inside
                # window qb itself), half=1 the rest.
                qs = slice(qb * P, (qb + 1) * P)
                cs = slice(half * 4, half * 4 + 4)
                pTf = ps.tile([P, 512], F32, tag="pz", bufs=4, name="pTf")
                pT = pTf[:, :].bitcast(BF)  # [P, 1024] bf16 view
                for ci in range(4):
                    c = half * 4 + ci
                    nc.tensor.transpose(
                        pT[:, ci * P : (ci + 1) * P],
                        z_tiles[qb][:, c * P : (c + 1) * P],
                        ident[:],
                    )
                if half == 0:
                    zTq = work.tile([P, EC, P], BF, tag="zt", bufs=1, name="zTq")
                    y_sb = work.tile([P, E], BF, tag="ysb", bufs=2, name="y_sb")
                    ln_state[qb] = [zTq, y_sb]
                zTq = ln_state[qb][0]
                nc.vector.tensor_tensor(
                    out=zTq[:, cs, :],
                    in0=pT[:, 0:512].rearrange("p (c q) -> p c q", q=P),
                    in1=xTq[:, cs, qs],
                    op=mybir.AluOpType.add,
                )

            def ln_proj(qb, fs):
                zTq, y_sb = ln_state[qb][:2]
                py = ps.tile([P, 512], F32, tag="pz", bufs=4, name="py")
                for c in range(EC):
                    nc.tensor.matmul(
                        py[:],
                        zTq[:, c, :],
                        Wp[fs][:, c, :],
                        start=(c == 0),
                        stop=(c == EC - 1),
                    )
                nc.vector.tensor_tensor(
                    out=y_sb[:, fs * 512 : (fs + 1) * 512],
                    in0=py[:],
                    in1=bp_bc[:, fs * 512 : (fs + 1) * 512],
                    op=mybir.AluOpType.add,
                )

            def ln_stats(qb):
                y_sb = ln_state[qb][1]
                sm = work.tile([P, 1], F32, tag="stat", bufs=16, name="sm")
                nc.vector.reduce_sum(sm[:], y_sb[:], axis=mybir.AxisListType.X)
                negmean = work.tile([P, 1], F32, tag="stat", bufs=16, name="nm")
                nc.vector.tensor_scalar_mul(negmean[:], sm[:], -inv_e)
                ysq = work.tile([P, E], BF, tag="yc", bufs=2, name="ysq")
                s2 = work.tile([P, 1], F32, tag="stat", bufs=16, name="s2")
                nc.vector.tensor_tensor(
                    out=ysq[:], in0=y_sb[:], in1=y_sb[:], op=mybir.AluOpType.mult
                )
                nc.vector.reduce_sum(s2[:], ysq[:], axis=mybir.AxisListType.X)
                nc.vector.tensor_scalar_mul(s2[:], s2[:], inv_e)
                # var = E[y^2] - mean^2 (+eps), rstd = 1/sqrt(var)
                mu2 = work.tile([P, 1], F32, tag="stat", bufs=16, name="mu2")
                nc.vector.tensor_tensor(
                    out=mu2[:], in0=negmean[:], in1=negmean[:],
                    op=mybir.AluOpType.mult,
                )
                nc.vector.tensor_scalar(
                    out=mu2[:], in0=mu2[:], scalar1=-1.0, scalar2=float(EPS),
                    op0=mybir.AluOpType.mult, op1=mybir.AluOpType.add,
                )
                var = work.tile([P, 1], F32, tag="stat", bufs=16, name="var")
                nc.vector.tensor_tensor(
                    out=var[:], in0=s2[:], in1=mu2[:], op=mybir.AluOpType.add
                )
                rstd = work.tile([P, 1], F32, tag="stat", bufs=16, name="rstd")
                nc.scalar.activation(
                    rstd[:], var[:], mybir.ActivationFunctionType.Sqrt
                )
                nc.vector.reciprocal(rstd[:], rstd[:])
                ln_state[qb] += [negmean, rstd]

            def ln_norm(qb):
                _, y_sb, negmean, rstd = ln_state.pop(qb)
                y_c = work.tile([P, E], BF, tag="yc", bufs=2, name="y_c")
                nc.vector.tensor_scalar(
                    out=y_c[:], in0=y_sb[:], scalar1=negmean[:, 0:1], scalar2=None,
                    op0=mybir.AluOpType.add,
                )
                nc.vector.tensor_tensor(
                    out=y_c[:], in0=y_c[:], in1=gamma_bc[:], op=mybir.AluOpType.mult
                )
                nc.vector.tensor_scalar(
                    out=y_c[:], in0=y_c[:], scalar1=rstd[:, 0:1], scalar2=None,
                    op0=mybir.AluOpType.mult,
                )
                y_f = work.tile([P, E], F32, tag="yf", bufs=1, name="y_f")
                nc.vector.tensor_tensor(
                    out=y_f[:], in0=y_c[:], in1=beta_bc[:], op=mybir.AluOpType.add
                )
                nc.sync.dma_start(y_d[qb, :, :], y_f[:])

            # ---- the fused attention loop ----
            # deferred work queue: each thunk is <=~1.5us of PE work; one is
            # drained per score group so injected work never starves the exp
            # pipeline. Thunks are tagged with their origin window; all
            # thunks from before the previous window are force-drained at
            # window boundaries to keep tile-rotation WARs sound.
            pending = []

            def drain(upto_window=None, limit=1):
                n = 0
                while pending and (
                    (upto_window is not None and pending[0][0] <= upto_window)
                    or (upto_window is None and n < limit)
                ):
                    pending.pop(0)[1]()
                    n += 1

            z_tiles = []
            z_cur = None
            pz_cur = None
            emit_kq_fb(0)
            prev_S = sgroup_for(0)
            for i, (u, g0, gw) in enumerate(flat):
                k_idx, pr, L = units[u]
                qs = slice(k_idx * P, (k_idx + 1) * P)
                h_e, h_o = 2 * pr, 2 * pr + 1
                if pr == 0 and g0 == 0:
                    drain(upto_window=k_idx - 1)
                    z_cur = work.tile([P, E], BF, tag="zsb", bufs=2, name="z_sb")
                    z_tiles.append(z_cur)
                if g0 == 0:
                    pz_cur = (
                        ps.tile([P, 512], F32, tag="pz", bufs=4, name="pE"),
                        ps.tile([P, 512], F32, tag="pz", bufs=4, name="pO"),
                    )
                pE, pO = pz_cur
                pS = prev_S
                w = gw * P
                eS = work.tile([P, 1024], BF, tag="eS", bufs=3, name="eS")
                nc.scalar.activation(
                    eS[:, :].rearrange("p (u q) -> p u q", u=2)[:, :, 0:w],
                    pS[:, :].rearrange("p (u q) -> p u q", u=2)[:, :, 0:w],
                    mybir.ActivationFunctionType.Exp,
                    scale=SCALE,
                )
                # enqueue deferred projection/LN work for later windows
                if g0 == 0:
                    if k_idx == 0 and pr + 1 < EC:
                        # K/Q chunks are consumed within this window: emit
                        # directly, not via the queue
                        emit_kq_fb(pr + 1)
                        if pr == 0:
                            emit_v_tb(0)
                            emit_v_tb(1)
                    if pr == 1 and 2 * k_idx + 2 < NB:
                        tb = 2 * k_idx + 2
                        pending.append((k_idx, lambda tb=tb: emit_v_tb(tb)))
                    if pr == 3 and 2 * k_idx + 3 < NB:
                        tb = 2 * k_idx + 3
                        pending.append((k_idx, lambda tb=tb: emit_v_tb(tb)))
                    if pr == 4:
                        # first half of this window's z is complete
                        pending.append((k_idx, lambda qb=k_idx: ln_tr(qb, 0)))
                    if k_idx >= 1:
                        qb = k_idx - 1
                        if pr == 0:
                            pending.append((k_idx, lambda qb=qb: ln_tr(qb, 1)))
                            pending.append((k_idx, lambda qb=qb: ln_proj(qb, 0)))
                        elif pr == 1:
                            pending.append((k_idx, lambda qb=qb: ln_proj(qb, 1)))
                        elif pr == 2:
                            pending.append((k_idx, lambda qb=qb: ln_stats(qb)))
                        elif pr == 3:
                            pending.append((k_idx, lambda qb=qb: ln_norm(qb)))
                if i + 1 < len(flat):
                    # next score group issues on PE while ACT runs this exp
                    prev_S = sgroup_for(i + 1)
                drain(limit=1)
                for jj in range(gw):
                    j = g0 + jj
                    if j >= L - 2:
                        m = m1 if j == L - 2 else m2
                        nc.gpsimd.tensor_tensor(
                            out=eS[:, :].rearrange("p (u q) -> p u q", u=2)[
                                :, :, jj * P : (jj + 1) * P
                            ],
                            in0=eS[:, :].rearrange("p (u q) -> p u q", u=2)[
                                :, :, jj * P : (jj + 1) * P
                            ],
                            in1=m[:, k_idx : k_idx + 1, :].to_broadcast((P, 2, P)),
                            op=mybir.AluOpType.mult,
                        )
                    for h, uu, zP in ((h_e, 0, pE), (h_o, 1, pO)):
                        nc.tensor.matmul(
                            zP[:, 0 : HD + 1],
                            eS[:, :].rearrange("p (u q) -> p u q", u=2)[
                                :, uu, jj * P : (jj + 1) * P
                            ],
                            Vx[:, j, h, :],
                            start=(j == 0),
                            stop=(j == L - 1),
                        )
                if g0 + gw == L:
                    # softmax division: denominator is per-partition (per-q)
                    for h, zP in ((h_e, pE), (h_o, pO)):
                        rs = work.tile([P, 1], F32, tag="rs", bufs=4, name="rs")
                        nc.vector.reciprocal(rs[:], zP[:, HD : HD + 1])
                        nc.vector.tensor_scalar(
                            out=z_cur[:, h * HD : (h + 1) * HD],
                            in0=zP[:, 0:HD],
                            scalar1=rs[:, 0:1],
                            scalar2=None,
                            op0=mybir.AluOpType.mult,
                        )
            # tail: drain leftovers and finish LN for the last query block
            drain(upto_window=NQ)
            ln_tr(NQ - 1, 1)
            ln_proj(NQ - 1, 0)
            ln_proj(NQ - 1, 1)
            ln_stats(NQ - 1)
            ln_norm(NQ - 1)

    _nc_cache["nc"] = nc
    return nc


def _make_masks(blocks):
    m1 = np.zeros((NQ, P, P), np.float32)
    m2 = np.zeros((NQ, P, P), np.float32)
    tril_t = (np.arange(P)[:, None] <= np.arange(P)[None, :]).astype(np.float32)
    for k in range(NQ):
        l_true = blocks[k] + 1
        L = PAD_L[k]
        if l_true == L:
            m1[k] = 1.0
            m2[k] = tril_t
        else:
            assert l_true == L - 1
            m1[k] = tril_t
            m2[k] = 0.0
    # device layout [P(k-local), NQ, P(q-local)]
    return (
        np.ascontiguousarray(m1.transpose(1, 0, 2)).astype(NPBF),
        np.ascontiguousarray(m2.transpose(1, 0, 2)).astype(NPBF),
    )


def kernel(x, Wq, bq, Wk, bk, Wv, bv, Wp, bp, gamma, beta):
    x = np.asarray(x, np.float32)
    nc = _build_nc()

    Wq8 = np.ascontiguousarray(np.asarray(Wq, np.float32).T).astype(NPF8)
    Wk8 = np.ascontiguousarray(np.asarray(Wk, np.float32).T).astype(NPF8)
    Wv8 = np.ascontiguousarray(np.asarray(Wv, np.float32).T).astype(NPF8)
    Wvr8 = (
        np.ascontiguousarray(np.asarray(Wv, np.float32).T)
        - Wv8.astype(np.float32)
    ).astype(NPF8)
    WpT = np.ascontiguousarray(np.asarray(Wp, np.float32).T).astype(NPBF)
    bqT = np.ascontiguousarray(np.asarray(bq, np.float32).reshape(EC, P).T)
    bkT = np.ascontiguousarray(np.asarray(bk, np.float32).reshape(EC, P).T)
    bv_bc = np.ascontiguousarray(
        np.broadcast_to(np.asarray(bv, np.float32), (P, E))
    ).astype(NPBF)
    bp_bc = np.ascontiguousarray(
        np.broadcast_to(np.asarray(bp, np.float32), (P, E))
    ).astype(NPBF)
    gamma_bc = np.ascontiguousarray(
        np.broadcast_to(np.asarray(gamma, np.float32), (P, E))
    ).astype(NPBF)
    beta_bc = np.ascontiguousarray(
        np.broadcast_to(np.asarray(beta, np.float32), (P, E))
    ).astype(NPBF)
    ident = np.eye(P, dtype=np.float32).astype(NPBF)
    masks = {0: _make_masks(BLOCKS_A), 1: _make_masks(BLOCKS_B)}

    in_maps = []
    for core in range(8):
        b, h = core // 2, core % 2
        blocks = BLOCKS_A if h == 0 else BLOCKS_B
        own = np.concatenate([np.arange(blk * P, (blk + 1) * P) for blk in blocks])
        xbT = np.ascontiguousarray(x[b].T)
        xT8_np = xbT.astype(NPF8)
        xr8_np = (
            xbT[:, 0 : 2 * P] - xT8_np[:, 0 : 2 * P].astype(np.float32)
        ).astype(NPF8)
        m1c, m2c = masks[h]
        in_maps.append(
            {
                "xT8": xT8_np,
                "xr8": xr8_np,
                "Wvr8": Wvr8,
                "xTq8": np.ascontiguousarray(xbT[:, own]).astype(NPF8),
                "xTq": np.ascontiguousarray(xbT[:, own]).astype(NPBF),
                "Wq8": Wq8,
                "Wk8": Wk8,
                "Wv8": Wv8,
                "WpT": WpT,
                "bqT": bqT,
                "bkT": bkT,
                "bv_bc": bv_bc,
                "bp_bc": bp_bc,
                "gamma_bc": gamma_bc,
                "beta_bc": beta_bc,
                "m1": m1c,
                "m2": m2c,
                "ident": ident,
            }
        )

    res = run_bass_kernel_spmd(nc, in_maps, core_ids=list(range(8)))

    out = np.empty((B, T, E), np.float32)
    for core in range(8):
        b, h = core // 2, core % 2
        blocks = BLOCKS_A if h == 0 else BLOCKS_B
        y = res.results[core]["y"]  # (NQ, P, E)
        for k, blk in enumerate(blocks):
            out[b, blk * P : (blk + 1) * P, :] = y[k]
    return out


# revision 48
# speedup vs baseline: 1.4784x; 1.0196x over previous
"""Multi-head self-attention (B=4, T=2048, E=1024, H=16) on 8 trn2 NeuronCores.

Sharding: core (b, h) = batch b, token-half h. Each core computes K/V for the
full sequence (duplicated within the batch pair), Q for its own 8 query blocks
of 128 tokens, causal attention for those blocks, then the output projection
and LayerNorm for its own tokens. Causal balance: query blocks are paired
(j, 15-j) so both cores process blocks with padded key-lengths 2,4,...,16;
host-supplied mask tiles encode the true causal structure, keeping the
compiled program identical across cores (SPMD).

Perf structure (cost-model driven). The kernel is one fused pipeline whose
rate limiter is the softmax exp on the ACT engine (~150us of the ~190us
total), so everything else is arranged to hide under it:
- Q/K/V projections run as fp8e4 DoubleRow matmuls (2x128 contraction per
  instruction at 0.5 cycles/row, 4x fewer PE cycles than bf16) and are
  emitted just-in-time inside the attention loop: K/Q per-head-pair chunks
  during the first query block, V blocks prefetched one window ahead.
- att@V uses exp-scores as the stationary operand and extended V (ones
  column for the softmax denominator) as the moving one, producing [q, d]
  output at 65 moving columns per instruction; the denominator lands in a
  per-partition scalar, so the division is reciprocal + one multiply.
- z is transposed back to [e, t] with PE transposes into a bf16-bitcast
  view of a f32 PSUM tile; the residual add is fused into the single drain.
- The projection + LayerNorm for query block q runs inside window q+1 of
  the attention loop, split into two stages so no engine queue blocks on
  the LN dependency chain; causal masks run on the idle GPSIMD engine.
- Input DMAs are issued alternately from the SP and ACT queues in
  criticality order (x, Wk, Wq, xq, masks, Wv, ...).
"""
import itertools
import json
import numpy as np
import ml_dtypes
from contextlib import ExitStack

import concourse.bass as bass
import concourse.bass_utils as _bass_utils
import concourse.tile as tile
from concourse import mybir
from concourse.bass_utils import run_bass_kernel_spmd

# ----------------------------------------------------------------------------
# Toolchain workarounds for this container's walrus build (see birfix notes):
# 1. EVENT_SEMAPHORE_RANGE_CLEAR InstISA is rejected ("ISA wrong length").
# 2. Engine instructions only carry one semaphore-wait slot; extra waits are
#    peeled onto NoOp carriers on the same engine (order-preserving).
# ----------------------------------------------------------------------------


def _patched_clear_and_free_semaphores(self, sems):
    if not sems:
        return
    sem_nums = [s.num if hasattr(s, "num") else s for s in sems]
    self._state.prepend_free_semaphores(sem_nums)
    for poison_set in self._tile_sem_poison_stack:
        poison_set.update(sem_nums)


def _fix_bir_waits(bir_json: bytes) -> bytes:
    bir = json.loads(bir_json)
    ctr = 0
    changed = False
    for func in bir.get("functions", []):
        for blk in func.get("blocks", []):
            out = []
            for inst in blk.get("instructions", []):
                si = inst.get("sync_info") or {}
                waits = si.get("on_wait") or []
                if len(waits) > 1:
                    for w in waits[:-1]:
                        ctr += 1
                        out.append(
                            {
                                "debug": inst.get("debug"),
                                "engine": inst.get("engine", "SP"),
                                "ins": [],
                                "name": f"IWF-{ctr}",
                                "opcode": "NoOp",
                                "outs": [],
                                "sync_info": {"on_wait": [w]},
                            }
                        )
                    si = dict(si)
                    si["on_wait"] = waits[-1:]
                    inst = dict(inst)
                    inst["sync_info"] = si
                    changed = True
                out.append(inst)
            blk["instructions"] = out
    return json.dumps(bir).encode() if changed else bir_json


_orig_compile_bir_kernel = _bass_utils.compile_bir_kernel


def _patched_compile_bir_kernel(bir_json, tmpdir, neff_name="file.neff"):
    if isinstance(bir_json, str):
        bir_json = bir_json.encode()
    return _orig_compile_bir_kernel(_fix_bir_waits(bir_json), tmpdir, neff_name)


def _install_patches():
    if getattr(bass.Bass, "_mhsa_patched", False):
        return
    bass.Bass.clear_and_free_semaphores = _patched_clear_and_free_semaphores
    bass.Bass._mhsa_patched = True
    _bass_utils.compile_bir_kernel = _patched_compile_bir_kernel
    try:
        import concourse.bass2jax as _b2j

        _b2j.compile_bir_kernel = _patched_compile_bir_kernel
    except ImportError:
        pass


_install_patches()

# ----------------------------------------------------------------------------
# Problem constants (hardcoded per spec)
# ----------------------------------------------------------------------------
B, T, E, H = 4, 2048, 1024, 16
HD = E // H  # 64
P = 128
NB = T // P  # 16 query/key blocks
NQ = 8  # query blocks per core
EC = E // P  # 8 e-chunks
SCALE = 1.0 / float(np.sqrt(T))
EPS = 1e-6
BF = mybir.dt.bfloat16
F32 = mybir.dt.float32
F8 = mybir.dt.float8e4
NPBF = ml_dtypes.bfloat16
NPF8 = ml_dtypes.float8_e4m3
DR = mybir.MatmulPerfMode.DoubleRow

# query-block assignment: pairs (j, 15-j); core h=0 takes even-j pairs' low
# and high ends so both cores see padded lengths L_k = 2(k+1)
BLOCKS_A = [0, 2, 4, 6, 9, 11, 13, 15]  # true lengths 1,3,5,7,10,12,14,16
BLOCKS_B = [1, 3, 5, 7, 8, 10, 12, 14]  # true lengths 2,4,6,8,9,11,13,15
PAD_L = [2 * (k + 1) for k in range(NQ)]  # 2,4,...,16

_nc_cache = {}


def _build_nc():
    if "nc" in _nc_cache:
        return _nc_cache["nc"]
    nc = bass.Bass(num_devices=8)

    # inputs (per-core)
    xT8_d = nc.dram_tensor("xT8", [E, T], F8, kind="ExternalInput")
    xTq8_d = nc.dram_tensor("xTq8", [E, NQ * P], F8, kind="ExternalInput")
    xTq_d = nc.dram_tensor("xTq", [E, NQ * P], BF, kind="ExternalInput")
    Wq8_d = nc.dram_tensor("Wq8", [E, E], F8, kind="ExternalInput")
    Wk8_d = nc.dram_tensor("Wk8", [E, E], F8, kind="ExternalInput")
    Wv8_d = nc.dram_tensor("Wv8", [E, E], F8, kind="ExternalInput")
    WpT_d = nc.dram_tensor("WpT", [E, E], BF, kind="ExternalInput")
    bqT_d = nc.dram_tensor("bqT", [P, EC], F32, kind="ExternalInput")
    bkT_d = nc.dram_tensor("bkT", [P, EC], F32, kind="ExternalInput")
    bv_d = nc.dram_tensor("bv_bc", [P, E], BF, kind="ExternalInput")
    bp_d = nc.dram_tensor("bp_bc", [P, E], BF, kind="ExternalInput")
    gm_d = nc.dram_tensor("gamma_bc", [P, E], BF, kind="ExternalInput")
    bt_d = nc.dram_tensor("beta_bc", [P, E], BF, kind="ExternalInput")
    m1_d = nc.dram_tensor("m1", [P, NQ, P], BF, kind="ExternalInput")
    m2_d = nc.dram_tensor("m2", [P, NQ, P], BF, kind="ExternalInput")
    id_d = nc.dram_tensor("ident", [P, P], BF, kind="ExternalInput")
    # fp8 second-digit residuals: fix the V projection for the first two
    # token blocks, where causal attention averages few values and fp8
    # quantization error would dominate the output absmax
    xr8_d = nc.dram_tensor("xr8", [E, 2 * P], F8, kind="ExternalInput")
    Wvr8_d = nc.dram_tensor("Wvr8", [E, E], F8, kind="ExternalInput")
    y_d = nc.dram_tensor("y", [NQ, P, E], F32, kind="ExternalOutput")

    with tile.TileContext(nc) as tc:
        with ExitStack() as ctx:
            consts = ctx.enter_context(tc.tile_pool(name="consts", bufs=1))
            big = ctx.enter_context(tc.tile_pool(name="big", bufs=1))
            wpool = ctx.enter_context(tc.tile_pool(name="wpool", bufs=1))
            work = ctx.enter_context(tc.tile_pool(name="work", bufs=2))
            ps = ctx.enter_context(tc.tile_pool(name="ps", bufs=1, space="PSUM"))

            # Critical loads alternate between the SP and ACT issue queues
            # (halving serial issue cost); tail loads go SP-only so the ACT
            # sequencer is free once the exp stream begins. Transfers are
            # chunked ~256KB so the 16 DMA engines start the K-projection
            # inputs within ~10us.
            _dma_rr = itertools.cycle([nc.sync, nc.scalar])

            def dma(dst, src):
                _dma_rr.__next__().dma_start(dst, src)

            def dma_sp(dst, src):
                nc.sync.dma_start(dst, src)

            # --- critical loads ---
            bkT = consts.tile([P, EC], F32)
            dma(bkT[:], bkT_d[:, :])
            bqT = consts.tile([P, EC], F32)
            dma(bqT[:], bqT_d[:, :])
            bv_bc = consts.tile([P, E], BF)
            dma(bv_bc[:], bv_d[:, :])
            xT8 = big.tile([P, EC, T], F8)
            for c in range(EC):
                dma(xT8[:, c, :], xT8_d.rearrange("(c p) t -> p c t", p=P)[:, c, :])

            def load_w8(dram, name, issue=dma):
                w = wpool.tile([P, EC, E], F8, tag="w8", bufs=4, name=name)
                for c0 in range(0, EC, 2):
                    issue(
                        w[:, c0 : c0 + 2, :],
                        dram.rearrange("(c p) f -> p c f", p=P)[:, c0 : c0 + 2, :],
                    )
                return w

            Wk = load_w8(Wk8_d, "Wk")
            Wq = load_w8(Wq8_d, "Wq")
            xTq8 = big.tile([P, EC, NQ * P], F8)
            for c0 in range(0, EC, 4):
                dma(
                    xTq8[:, c0 : c0 + 4, :],
                    xTq8_d.rearrange("(c p) t -> p c t", p=P)[:, c0 : c0 + 4, :],
                )
            Wv = load_w8(Wv8_d, "Wv")
            Wvr = load_w8(Wvr8_d, "Wvr")
            xr8 = big.tile([P, EC, 2 * P], F8)
            dma(xr8[:], xr8_d.rearrange("(c p) t -> p c t", p=P)[:, :, :])
            m1 = consts.tile([P, NQ, P], BF)
            dma(m1[:], m1_d[:, :, :])
            m2 = consts.tile([P, NQ, P], BF)
            dma(m2[:], m2_d[:, :, :])
            ident = consts.tile([P, P], BF)
            dma(ident[:], id_d[:, :])
            # --- tail loads (SP queue only) ---
            xTq = big.tile([P, EC, NQ * P], BF)
            for c0 in range(0, EC, 2):
                dma_sp(
                    xTq[:, c0 : c0 + 2, :],
                    xTq_d.rearrange("(c p) t -> p c t", p=P)[:, c0 : c0 + 2, :],
                )
            Wp = []
            for hf in range(2):
                wph = wpool.tile(
                    [P, EC, E // 2], BF, tag="wp", bufs=2, name=f"Wp{hf}"
                )
                for c0 in range(0, EC, 2):
                    dma_sp(
                        wph[:, c0 : c0 + 2, :],
                        WpT_d.rearrange("(c p) f -> p c f", p=P)[
                            :, c0 : c0 + 2, hf * 512 : (hf + 1) * 512
                        ],
                    )
                Wp.append(wph)
            bp_bc = consts.tile([P, E], BF)
            dma_sp(bp_bc[:], bp_d[:, :])
            gamma_bc = consts.tile([P, E], BF)
            dma_sp(gamma_bc[:], gm_d[:, :])
            beta_bc = consts.tile([P, E], BF)
            dma_sp(beta_bc[:], bt_d[:, :])

            # persistent intermediates
            KT = big.tile([P, EC, T], BF)  # K^T  [f, t]
            QT = big.tile([P, EC, NQ * P], BF)  # Q^T  [f, t_own]
            Vx = big.tile([P, NB, H, HD + 1], BF)  # V ext [t, h, d|1]
            nc.vector.memset(Vx[:, :, :, HD : HD + 1], 1.0)

            inv_e = 1.0 / float(E)

            # ---- fp8 DoubleRow projection emitters ----
            def emit_k_fb(fb, t2):
                # K^T rows for feature chunk fb, tokens t2*512..(t2+1)*512
                pk = ps.tile([P, 512], F32, tag="pz", bufs=4, name="pk")
                for half in range(2):
                    ts_ = slice((t2 * 2 + half) * 256, (t2 * 2 + half) * 256 + 256)
                    for cg in range(4):
                        nc.tensor.matmul(
                            pk[:, half * 256 : half * 256 + 256],
                            Wk[:, 2 * cg : 2 * cg + 2, fb * P : (fb + 1) * P],
                            xT8[:, 2 * cg : 2 * cg + 2, ts_],
                            start=(cg == 0),
                            stop=(cg == 3),
                            perf_mode=DR,
                        )
                nc.vector.tensor_scalar(
                    out=KT[:, fb, t2 * 512 : (t2 + 1) * 512],
                    in0=pk[:],
                    scalar1=bkT[:, fb : fb + 1],
                    scalar2=None,
                    op0=mybir.AluOpType.add,
                )

            def emit_q_fb(fb, blk):
                # Q^T rows for feature chunk fb, own query block blk
                qs = slice(blk * P, (blk + 1) * P)
                pq = ps.tile([P, 512], F32, tag="pz", bufs=4, name="pq")
                for cg in range(4):
                    nc.tensor.matmul(
                        pq[:, 0:P],
                        Wq[:, 2 * cg : 2 * cg + 2, fb * P : (fb + 1) * P],
                        xTq8[:, 2 * cg : 2 * cg + 2, qs],
                        start=(cg == 0),
                        stop=(cg == 3),
                        perf_mode=DR,
                    )
                nc.vector.tensor_scalar(
                    out=QT[:, fb, qs],
                    in0=pq[:, 0:P],
                    scalar1=bqT[:, fb : fb + 1],
                    scalar2=None,
                    op0=mybir.AluOpType.add,
                )

            def emit_v_tb(tb):
                # V rows for token block tb; first two blocks get a
                # first-order fp8 residual correction (x_r@Wv + x@Wv_r)
                passes = [(xT8, Wv)]
                if tb < 2:
                    passes += [(xr8, Wv), (xT8, Wvr)]
                for f2 in range(2):
                    pv = ps.tile([P, 512], F32, tag="pz", bufs=4, name="pv")
                    for half in range(2):
                        fs = slice((f2 * 2 + half) * 256, (f2 * 2 + half) * 256 + 256)
                        for pi, (xs, ws) in enumerate(passes):
                            xcols = slice(tb * P, (tb + 1) * P)
                            for cg in range(4):
                                nc.tensor.matmul(
                                    pv[:, half * 256 : half * 256 + 256],
                                    xs[:, 2 * cg : 2 * cg + 2, xcols],
                                    ws[:, 2 * cg : 2 * cg + 2, fs],
                                    start=(pi == 0 and cg == 0),
                                    stop=(pi == len(passes) - 1 and cg == 3),
                                    perf_mode=DR,
                                )
                    nc.vector.tensor_tensor(
                        out=Vx[:, tb, f2 * 8 : (f2 + 1) * 8, 0:HD],
                        in0=pv[:, :].rearrange("p (h d) -> p h d", d=HD),
                        in1=bv_bc[:, f2 * 512 : (f2 + 1) * 512].rearrange(
                            "p (h d) -> p h d", d=HD
                        ),
                        op=mybir.AluOpType.add,
                    )

            # ---- attention score group ----
            def emit_sgroup(pr, qs, g0, gw):
                pS = ps.tile([P, 1024], F32, tag="S", bufs=2, name="pS")
                for jj in range(gw):
                    js = slice((g0 + jj) * P, (g0 + jj + 1) * P)
                    nc.tensor.matmul(
                        pS[:, jj * P : (jj + 1) * P],
                        KT[0:64, pr, js],
                        QT[0:64, pr, qs],
                        start=True,
                        stop=True,
                        tile_position=(0, 0),
                    )
                    nc.tensor.matmul(
                        pS[:, 512 + jj * P : 512 + (jj + 1) * P],
                        KT[64:128, pr, js],
                        QT[64:128, pr, qs],
                        start=True,
                        stop=True,
                        tile_position=(64, 0),
                    )
                return pS

            units = []
            flat = []
            for k_idx in range(NQ):
                L = PAD_L[k_idx]
                for pr in range(H // 2):
                    u = len(units)
                    units.append((k_idx, pr, L))
                    for g0 in range(0, L, 4):
                        flat.append((u, g0, min(4, L - g0)))

            def sgroup_for(idx):
                u, g0, gw = flat[idx]
                k_idx, pr, L = units[u]
                return emit_sgroup(pr, slice(k_idx * P, (k_idx + 1) * P), g0, gw)

            # ---- projection + LN pipeline (runs inside window qb+1),
            # split into small thunks so injected PE work never delays the
            # score pipeline by more than ~1us ----
            ln_state = {}

            def ln_tr(qb, half):
                # transpose z[q, e] -> [e, q] via PE into a bf16 view of a
                # f32 psum tile; drain fuses the residual add. half=0 covers
                # heads 0-7 (available right after pr3's division, inside
                # window qb itself), half=1 the rest.
                qs = slice(qb * P, (qb + 1) * P)
                cs = slice(half * 4, half * 4 + 4)
                pTf = ps.tile([P, 512], F32, tag="pz", bufs=4, name="pTf")
                pT = pTf[:, :].bitcast(BF)  # [P, 1024] bf16 view
                for ci in range(4):
                    c = half * 4 + ci
                    nc.tensor.transpose(
                        pT[:, ci * P : (ci + 1) * P],
                        z_tiles[qb][:, c * P : (c + 1) * P],
                        ident[:],
                    )
                if half == 0:
                    zTq = work.tile([P, EC, P], BF, tag="zt", bufs=1, name="zTq")
                    y_sb = work.tile([P, E], BF, tag="ysb", bufs=2, name="y_sb")
                    ln_state[qb] = [zTq, y_sb]
                zTq = ln_state[qb][0]
                nc.vector.tensor_tensor(
                    out=zTq[:, cs, :],
                    in0=pT[:, 0:512].rearrange("p (c q) -> p c q", q=P),
                    in1=xTq[:, cs, qs],
                    op=mybir.AluOpType.add,
                )

            def ln_proj(qb, fs):
                zTq, y_sb = ln_state[qb][:2]
                py = ps.tile([P, 512], F32, tag="pz", bufs=4, name="py")
                for c in range(EC):
                    nc.tensor.matmul(
                        py[:],
                        zTq[:, c, :],
                        Wp[fs][:, c, :],
                        start=(c == 0),
                        stop=(c == EC - 1),
                    )
                nc.vector.tensor_tensor(
                    out=y_sb[:, fs * 512 : (fs + 1) * 512],
                    in0=py[:],
                    in1=bp_bc[:, fs * 512 : (fs + 1) * 512],
                    op=mybir.AluOpType.add,
                )

            def ln_stats(qb):
                y_sb = ln_state[qb][1]
                sm = work.tile([P, 1], F32, tag="stat", bufs=16, name="sm")
                nc.vector.reduce_sum(sm[:], y_sb[:], axis=mybir.AxisListType.X)
                negmean = work.tile([P, 1], F32, tag="stat", bufs=16, name="nm")
                nc.vector.tensor_scalar_mul(negmean[:], sm[:], -inv_e)
                ysq = work.tile([P, E], BF, tag="yc", bufs=2, name="ysq")
                s2 = work.tile([P, 1], F32, tag="stat", bufs=16, name="s2")
                nc.vector.tensor_tensor(
                    out=ysq[:], in0=y_sb[:], in1=y_sb[:], op=mybir.AluOpType.mult
                )
                nc.vector.reduce_sum(s2[:], ysq[:], axis=mybir.AxisListType.X)
                nc.vector.tensor_scalar_mul(s2[:], s2[:], inv_e)
                # var = E[y^2] - mean^2 (+eps), rstd = 1/sqrt(var)
                mu2 = work.tile([P, 1], F32, tag="stat", bufs=16, name="mu2")
                nc.vector.tensor_tensor(
                    out=mu2[:], in0=negmean[:], in1=negmean[:],
                    op=mybir.AluOpType.mult,
                )
                nc.vector.tensor_scalar(
                    out=mu2[:], in0=mu2[:], scalar1=-1.0, scalar2=float(EPS),
                    op0=mybir.AluOpType.mult, op1=mybir.AluOpType.add,
                )
                var = work.tile([P, 1], F32, tag="stat", bufs=16, name="var")
                nc.vector.tensor_tensor(
                    out=var[:], in0=s2[:], in1=mu2[:], op=mybir.AluOpType.add
                )
                rstd = work.tile([P, 1], F32, tag="stat", bufs=16, name="rstd")
                nc.scalar.activation(
                    rstd[:], var[:], mybir.ActivationFunctionType.Sqrt
                )
                nc.vector.reciprocal(rstd[:], rstd[:])
                ln_state[qb] += [negmean, rstd]

            def ln_norm(qb):
                _, y_sb, negmean, rstd = ln_state.pop(qb)
                y_c = work.tile([P, E], BF, tag="yc", bufs=2, name="y_c")
                nc.vector.tensor_scalar(
                    out=y_c[:], in0=y_sb[:], scalar1=negmean[:, 0:1], scalar2=None,
                    op0=mybir.AluOpType.add,
                )
                nc.vector.tensor_tensor(
                    out=y_c[:], in0=y_c[:], in1=gamma_bc[:], op=mybir.AluOpType.mult
                )
                nc.vector.tensor_scalar(
                    out=y_c[:], in0=y_c[:], scalar1=rstd[:, 0:1], scalar2=None,
                    op0=mybir.AluOpType.mult,
                )
                y_f = work.tile([P, E], F32, tag="yf", bufs=1, name="y_f")
                nc.vector.tensor_tensor(
                    out=y_f[:], in0=y_c[:], in1=beta_bc[:], op=mybir.AluOpType.add
                )
                nc.sync.dma_start(y_d[qb, :, :], y_f[:])

            # ---- the fused attention loop ----
            # deferred work queue: each thunk is <=~1.5us of PE work; one is
            # drained per score group so injected work never starves the exp
            # pipeline. Thunks are tagged with their origin window; all
            # thunks from before the previous window are force-drained at
            # window boundaries to keep tile-rotation WARs sound.
            pending = []

            def drain(upto_window=None, limit=1):
                n = 0
                while pending and (
                    (upto_window is not None and pending[0][0] <= upto_window)
                    or (upto_window is None and n < limit)
                ):
                    pending.pop(0)[1]()
                    n += 1

            z_tiles = []
            z_cur = None
            pz_cur = None
            emit_k_fb(0, 0)
            emit_q_fb(0, 0)
            prev_S = sgroup_for(0)
            for i, (u, g0, gw) in enumerate(flat):
                k_idx, pr, L = units[u]
                qs = slice(k_idx * P, (k_idx + 1) * P)
                h_e, h_o = 2 * pr, 2 * pr + 1
                if pr == 0 and g0 == 0:
                    drain(upto_window=k_idx - 1)
                    z_cur = work.tile([P, E], BF, tag="zsb", bufs=2, name="z_sb")
                    z_tiles.append(z_cur)
                if g0 == 0:
                    pz_cur = (
                        ps.tile([P, 512], F32, tag="pz", bufs=4, name="pE"),
                        ps.tile([P, 512], F32, tag="pz", bufs=4, name="pO"),
                    )
                pE, pO = pz_cur
                pS = prev_S
                w = gw * P
                eS = work.tile([P, 1024], BF, tag="eS", bufs=3, name="eS")
                nc.scalar.activation(
                    eS[:, :].rearrange("p (u q) -> p u q", u=2)[:, :, 0:w],
                    pS[:, :].rearrange("p (u q) -> p u q", u=2)[:, :, 0:w],
                    mybir.ActivationFunctionType.Exp,
                    scale=SCALE,
                )
                # enqueue deferred projection/LN work for later windows
                if g0 == 0:
                    if k_idx == 0 and pr + 1 < EC:
                        # K/Q chunks consumed within this window: emit
                        # directly, not via the queue
                        emit_k_fb(pr + 1, 0)
                        emit_q_fb(pr + 1, 0)
                        if pr == 0:
                            emit_v_tb(0)
                            emit_v_tb(1)
                    if pr == 1 and 2 * k_idx + 2 < NB:
                        tb = 2 * k_idx + 2
                        pending.append((k_idx, lambda tb=tb: emit_v_tb(tb)))
                    if pr == 3 and 2 * k_idx + 3 < NB:
                        tb = 2 * k_idx + 3
                        pending.append((k_idx, lambda tb=tb: emit_v_tb(tb)))
                    if pr == 2 and k_idx < NQ - 1:
                        # Q rows for the next window's query block
                        blk = k_idx + 1

                        def _q(blk=blk):
                            for fb in range(EC):
                                emit_q_fb(fb, blk)

                        pending.append((k_idx, _q))
                    if pr == 5 and k_idx in (0, 2, 4):
                        # K token chunks ahead of the window that needs them
                        t2 = k_idx // 2 + 1
                        for fb in range(EC):
                            pending.append(
                                (k_idx + 1, lambda fb=fb, t2=t2: emit_k_fb(fb, t2))
                            )
                    if pr == 4:
                        # first half of this window's z is complete
                        pending.append((k_idx, lambda qb=k_idx: ln_tr(qb, 0)))
                    if k_idx >= 1:
                        qb = k_idx - 1
                        if pr == 0:
                            pending.append((k_idx, lambda qb=qb: ln_tr(qb, 1)))
                            pending.append((k_idx, lambda qb=qb: ln_proj(qb, 0)))
                        elif pr == 1:
                            pending.append((k_idx, lambda qb=qb: ln_proj(qb, 1)))
                        elif pr == 2:
                            pending.append((k_idx, lambda qb=qb: ln_stats(qb)))
                        elif pr == 3:
                            pending.append((k_idx, lambda qb=qb: ln_norm(qb)))
                if i + 1 < len(flat):
                    # next score group issues on PE while ACT runs this exp;
                    # at window transitions, first force-drain everything the
                    # next window's score groups may read
                    nk = units[flat[i + 1][0]][0]
                    if nk != k_idx:
                        drain(upto_window=nk - 1)
                    prev_S = sgroup_for(i + 1)
                drain(limit=1)
                for jj in range(gw):
                    j = g0 + jj
                    if j >= L - 2:
                        m = m1 if j == L - 2 else m2
                        nc.gpsimd.tensor_tensor(
                            out=eS[:, :].rearrange("p (u q) -> p u q", u=2)[
                                :, :, jj * P : (jj + 1) * P
                            ],
                            in0=eS[:, :].rearrange("p (u q) -> p u q", u=2)[
                                :, :, jj * P : (jj + 1) * P
                            ],
                            in1=m[:, k_idx : k_idx + 1, :].to_broadcast((P, 2, P)),
                            op=mybir.AluOpType.mult,
                        )
                    for h, uu, zP in ((h_e, 0, pE), (h_o, 1, pO)):
                        nc.tensor.matmul(
                            zP[:, 0 : HD + 1],
                            eS[:, :].rearrange("p (u q) -> p u q", u=2)[
                                :, uu, jj * P : (jj + 1) * P
                            ],
                            Vx[:, j, h, :],
                            start=(j == 0),
                            stop=(j == L - 1),
                        )
                if g0 + gw == L:
                    # softmax division: denominator is per-partition (per-q)
                    for h, zP in ((h_e, pE), (h_o, pO)):
                        rs = work.tile([P, 1], F32, tag="rs", bufs=4, name="rs")
                        nc.vector.reciprocal(rs[:], zP[:, HD : HD + 1])
                        nc.vector.tensor_scalar(
                            out=z_cur[:, h * HD : (h + 1) * HD],
                            in0=zP[:, 0:HD],
                            scalar1=rs[:, 0:1],
                            scalar2=None,
                            op0=mybir.AluOpType.mult,
                        )
            # tail: drain leftovers and finish LN for the last query block
            drain(upto_window=NQ)
            ln_tr(NQ - 1, 1)
            ln_proj(NQ - 1, 0)
            ln_proj(NQ - 1, 1)
            ln_stats(NQ - 1)
            ln_norm(NQ - 1)

    _nc_cache["nc"] = nc
    return nc


def _make_masks(blocks):
    m1 = np.zeros((NQ, P, P), np.float32)
    m2 = np.zeros((NQ, P, P), np.float32)
    tril_t = (np.arange(P)[:, None] <= np.arange(P)[None, :]).astype(np.float32)
    for k in range(NQ):
        l_true = blocks[k] + 1
        L = PAD_L[k]
        if l_true == L:
            m1[k] = 1.0
            m2[k] = tril_t
        else:
            assert l_true == L - 1
            m1[k] = tril_t
            m2[k] = 0.0
    # device layout [P(k-local), NQ, P(q-local)]
    return (
        np.ascontiguousarray(m1.transpose(1, 0, 2)).astype(NPBF),
        np.ascontiguousarray(m2.transpose(1, 0, 2)).astype(NPBF),
    )


def kernel(x, Wq, bq, Wk, bk, Wv, bv, Wp, bp, gamma, beta):
    x = np.asarray(x, np.float32)
    nc = _build_nc()

    Wq8 = np.ascontiguousarray(np.asarray(Wq, np.float32).T).astype(NPF8)
    Wk8 = np.ascontiguousarray(np.asarray(Wk, np.float32).T).astype(NPF8)
    Wv8 = np.ascontiguousarray(np.asarray(Wv, np.float32).T).astype(NPF8)
    Wvr8 = (
        np.ascontiguousarray(np.asarray(Wv, np.float32).T)
        - Wv8.astype(np.float32)
    ).astype(NPF8)
    WpT = np.ascontiguousarray(np.asarray(Wp, np.float32).T).astype(NPBF)
    bqT = np.ascontiguousarray(np.asarray(bq, np.float32).reshape(EC, P).T)
    bkT = np.ascontiguousarray(np.asarray(bk, np.float32).reshape(EC, P).T)
    bv_bc = np.ascontiguousarray(
        np.broadcast_to(np.asarray(bv, np.float32), (P, E))
    ).astype(NPBF)
    bp_bc = np.ascontiguousarray(
        np.broadcast_to(np.asarray(bp, np.float32), (P, E))
    ).astype(NPBF)
    gamma_bc = np.ascontiguousarray(
        np.broadcast_to(np.asarray(gamma, np.float32), (P, E))
    ).astype(NPBF)
    beta_bc = np.ascontiguousarray(
        np.broadcast_to(np.asarray(beta, np.float32), (P, E))
    ).astype(NPBF)
    ident = np.eye(P, dtype=np.float32).astype(NPBF)
    masks = {0: _make_masks(BLOCKS_A), 1: _make_masks(BLOCKS_B)}

    in_maps = []
    for core in range(8):
        b, h = core // 2, core % 2
        blocks = BLOCKS_A if h == 0 else BLOCKS_B
        own = np.concatenate([np.arange(blk * P, (blk + 1) * P) for blk in blocks])
        xbT = np.ascontiguousarray(x[b].T)
        xT8_np = xbT.astype(NPF8)
        xr8_np = (
            xbT[:, 0 : 2 * P] - xT8_np[:, 0 : 2 * P].astype(np.float32)
        ).astype(NPF8)
        m1c, m2c = masks[h]
        in_maps.append(
            {
                "xT8": xT8_np,
                "xr8": xr8_np,
                "Wvr8": Wvr8,
                "xTq8": np.ascontiguousarray(xbT[:, own]).astype(NPF8),
                "xTq": np.ascontiguousarray(xbT[:, own]).astype(NPBF),
                "Wq8": Wq8,
                "Wk8": Wk8,
                "Wv8": Wv8,
                "WpT": WpT,
                "bqT": bqT,
                "bkT": bkT,
                "bv_bc": bv_bc,
                "bp_bc": bp_bc,
                "gamma_bc": gamma_bc,
                "beta_bc": beta_bc,
                "m1": m1c,
                "m2": m2c,
                "ident": ident,
            }
        )

    res = run_bass_kernel_spmd(nc, in_maps, core_ids=list(range(8)))

    out = np.empty((B, T, E), np.float32)
    for core in range(8):
        b, h = core // 2, core % 2
        blocks = BLOCKS_A if h == 0 else BLOCKS_B
        y = res.results[core]["y"]  # (NQ, P, E)
        for k, blk in enumerate(blocks):
            out[b, blk * P : (blk + 1) * P, :] = y[k]
    return out


# revision 51
# speedup vs baseline: 1.4821x; 1.0024x over previous
"""Multi-head self-attention (B=4, T=2048, E=1024, H=16) on 8 trn2 NeuronCores.

Sharding: core (b, h) = batch b, token-half h. Each core computes K/V for the
full sequence (duplicated within the batch pair), Q for its own 8 query blocks
of 128 tokens, causal attention for those blocks, then the output projection
and LayerNorm for its own tokens. Causal balance: query blocks are paired
(j, 15-j) so both cores process blocks with padded key-lengths 2,4,...,16;
host-supplied mask tiles encode the true causal structure, keeping the
compiled program identical across cores (SPMD).

Perf structure (cost-model driven). The kernel is one fused pipeline whose
rate limiter is the softmax exp on the ACT engine (~150us of the ~190us
total), so everything else is arranged to hide under it:
- Q/K/V projections run as fp8e4 DoubleRow matmuls (2x128 contraction per
  instruction at 0.5 cycles/row, 4x fewer PE cycles than bf16) and are
  emitted just-in-time inside the attention loop: K/Q per-head-pair chunks
  during the first query block, V blocks prefetched one window ahead.
- att@V uses exp-scores as the stationary operand and extended V (ones
  column for the softmax denominator) as the moving one, producing [q, d]
  output at 65 moving columns per instruction; the denominator lands in a
  per-partition scalar, so the division is reciprocal + one multiply.
- z is transposed back to [e, t] with PE transposes into a bf16-bitcast
  view of a f32 PSUM tile; the residual add is fused into the single drain.
- The projection + LayerNorm for query block q runs inside window q+1 of
  the attention loop, split into two stages so no engine queue blocks on
  the LN dependency chain; causal masks run on the idle GPSIMD engine.
- Input DMAs are issued alternately from the SP and ACT queues in
  criticality order (x, Wk, Wq, xq, masks, Wv, ...).
"""
import itertools
import json
import numpy as np
import ml_dtypes
from contextlib import ExitStack

import concourse.bass as bass
import concourse.bass_utils as _bass_utils
import concourse.tile as tile
from concourse import mybir
from concourse.bass_utils import run_bass_kernel_spmd

# ----------------------------------------------------------------------------
# Toolchain workarounds for this container's walrus build (see birfix notes):
# 1. EVENT_SEMAPHORE_RANGE_CLEAR InstISA is rejected ("ISA wrong length").
# 2. Engine instructions only carry one semaphore-wait slot; extra waits are
#    peeled onto NoOp carriers on the same engine (order-preserving).
# ----------------------------------------------------------------------------


def _patched_clear_and_free_semaphores(self, sems):
    if not sems:
        return
    sem_nums = [s.num if hasattr(s, "num") else s for s in sems]
    self._state.prepend_free_semaphores(sem_nums)
    for poison_set in self._tile_sem_poison_stack:
        poison_set.update(sem_nums)


def _fix_bir_waits(bir_json: bytes) -> bytes:
    bir = json.loads(bir_json)
    ctr = 0
    changed = False
    for func in bir.get("functions", []):
        for blk in func.get("blocks", []):
            out = []
            for inst in blk.get("instructions", []):
                si = inst.get("sync_info") or {}
                waits = si.get("on_wait") or []
                if len(waits) > 1:
                    for w in waits[:-1]:
                        ctr += 1
                        out.append(
                            {
                                "debug": inst.get("debug"),
                                "engine": inst.get("engine", "SP"),
                                "ins": [],
                                "name": f"IWF-{ctr}",
                                "opcode": "NoOp",
                                "outs": [],
                                "sync_info": {"on_wait": [w]},
                            }
                        )
                    si = dict(si)
                    si["on_wait"] = waits[-1:]
                    inst = dict(inst)
                    inst["sync_info"] = si
                    changed = True
                out.append(inst)
            blk["instructions"] = out
    return json.dumps(bir).encode() if changed else bir_json


_orig_compile_bir_kernel = _bass_utils.compile_bir_kernel


def _patched_compile_bir_kernel(bir_json, tmpdir, neff_name="file.neff"):
    if isinstance(bir_json, str):
        bir_json = bir_json.encode()
    return _orig_compile_bir_kernel(_fix_bir_waits(bir_json), tmpdir, neff_name)


def _install_patches():
    if getattr(bass.Bass, "_mhsa_patched", False):
        return
    bass.Bass.clear_and_free_semaphores = _patched_clear_and_free_semaphores
    bass.Bass._mhsa_patched = True
    _bass_utils.compile_bir_kernel = _patched_compile_bir_kernel
    try:
        import concourse.bass2jax as _b2j

        _b2j.compile_bir_kernel = _patched_compile_bir_kernel
    except ImportError:
        pass


_install_patches()

# ----------------------------------------------------------------------------
# Problem constants (hardcoded per spec)
# ----------------------------------------------------------------------------
B, T, E, H = 4, 2048, 1024, 16
HD = E // H  # 64
P = 128
NB = T // P  # 16 query/key blocks
NQ = 8  # query blocks per core
EC = E // P  # 8 e-chunks
SCALE = 1.0 / float(np.sqrt(T))
EPS = 1e-6
BF = mybir.dt.bfloat16
F32 = mybir.dt.float32
F8 = mybir.dt.float8e4
NPBF = ml_dtypes.bfloat16
NPF8 = ml_dtypes.float8_e4m3
DR = mybir.MatmulPerfMode.DoubleRow

# query-block assignment: pairs (j, 15-j); core h=0 takes even-j pairs' low
# and high ends so both cores see padded lengths L_k = 2(k+1)
BLOCKS_A = [0, 2, 4, 6, 9, 11, 13, 15]  # true lengths 1,3,5,7,10,12,14,16
BLOCKS_B = [1, 3, 5, 7, 8, 10, 12, 14]  # true lengths 2,4,6,8,9,11,13,15
PAD_L = [2 * (k + 1) for k in range(NQ)]  # 2,4,...,16

_nc_cache = {}


def _build_nc():
    if "nc" in _nc_cache:
        return _nc_cache["nc"]
    nc = bass.Bass(num_devices=8)

    # inputs (per-core)
    xT8_d = nc.dram_tensor("xT8", [E, T], F8, kind="ExternalInput")
    xTq8_d = nc.dram_tensor("xTq8", [E, NQ * P], F8, kind="ExternalInput")
    xTq_d = nc.dram_tensor("xTq", [E, NQ * P], BF, kind="ExternalInput")
    Wq8_d = nc.dram_tensor("Wq8", [E, E], F8, kind="ExternalInput")
    Wk8_d = nc.dram_tensor("Wk8", [E, E], F8, kind="ExternalInput")
    Wv8_d = nc.dram_tensor("Wv8", [E, E], F8, kind="ExternalInput")
    WpT_d = nc.dram_tensor("WpT", [E, E], BF, kind="ExternalInput")
    bqT_d = nc.dram_tensor("bqT", [P, EC], F32, kind="ExternalInput")
    bkT_d = nc.dram_tensor("bkT", [P, EC], F32, kind="ExternalInput")
    bv_d = nc.dram_tensor("bv_bc", [P, E], BF, kind="ExternalInput")
    bp_d = nc.dram_tensor("bp_bc", [P, E], BF, kind="ExternalInput")
    gm_d = nc.dram_tensor("gamma_bc", [P, E], BF, kind="ExternalInput")
    bt_d = nc.dram_tensor("beta_bc", [P, E], BF, kind="ExternalInput")
    m1_d = nc.dram_tensor("m1", [P, NQ, P], BF, kind="ExternalInput")
    m2_d = nc.dram_tensor("m2", [P, NQ, P], BF, kind="ExternalInput")
    id_d = nc.dram_tensor("ident", [P, P], BF, kind="ExternalInput")
    # fp8 second-digit residuals: fix the V projection for the first two
    # token blocks, where causal attention averages few values and fp8
    # quantization error would dominate the output absmax
    xr8_d = nc.dram_tensor("xr8", [E, 2 * P], F8, kind="ExternalInput")
    Wvr8_d = nc.dram_tensor("Wvr8", [E, E], F8, kind="ExternalInput")
    y_d = nc.dram_tensor("y", [NQ, P, E], F32, kind="ExternalOutput")

    with tile.TileContext(nc) as tc:
        with ExitStack() as ctx:
            consts = ctx.enter_context(tc.tile_pool(name="consts", bufs=1))
            big = ctx.enter_context(tc.tile_pool(name="big", bufs=1))
            wpool = ctx.enter_context(tc.tile_pool(name="wpool", bufs=1))
            work = ctx.enter_context(tc.tile_pool(name="work", bufs=2))
            ps = ctx.enter_context(tc.tile_pool(name="ps", bufs=1, space="PSUM"))

            # Critical loads alternate between the SP and ACT issue queues
            # (halving serial issue cost); tail loads go SP-only so the ACT
            # sequencer is free once the exp stream begins. Transfers are
            # chunked ~256KB so the 16 DMA engines start the K-projection
            # inputs within ~10us.
            _dma_rr = itertools.cycle([nc.sync, nc.scalar])

            def dma(dst, src):
                _dma_rr.__next__().dma_start(dst, src)

            def dma_sp(dst, src):
                nc.sync.dma_start(dst, src)

            # --- critical loads ---
            bkT = consts.tile([P, EC], F32)
            dma(bkT[:], bkT_d[:, :])
            bqT = consts.tile([P, EC], F32)
            dma(bqT[:], bqT_d[:, :])
            bv_bc = consts.tile([P, E], BF)
            dma(bv_bc[:], bv_d[:, :])
            xT8 = big.tile([P, EC, T], F8)
            for c in range(EC):
                dma(xT8[:, c, :], xT8_d.rearrange("(c p) t -> p c t", p=P)[:, c, :])

            def load_w8(dram, name, issue=dma):
                w = wpool.tile([P, EC, E], F8, tag="w8", bufs=4, name=name)
                for c0 in range(0, EC, 2):
                    issue(
                        w[:, c0 : c0 + 2, :],
                        dram.rearrange("(c p) f -> p c f", p=P)[:, c0 : c0 + 2, :],
                    )
                return w

            Wk = load_w8(Wk8_d, "Wk")
            Wq = load_w8(Wq8_d, "Wq")
            xTq8 = big.tile([P, EC, NQ * P], F8)
            for c0 in range(0, EC, 4):
                dma(
                    xTq8[:, c0 : c0 + 4, :],
                    xTq8_d.rearrange("(c p) t -> p c t", p=P)[:, c0 : c0 + 4, :],
                )
            Wv = load_w8(Wv8_d, "Wv")
            Wvr = load_w8(Wvr8_d, "Wvr")
            xr8 = big.tile([P, EC, 2 * P], F8)
            dma(xr8[:], xr8_d.rearrange("(c p) t -> p c t", p=P)[:, :, :])
            m1 = consts.tile([P, NQ, P], BF)
            dma(m1[:], m1_d[:, :, :])
            m2 = consts.tile([P, NQ, P], BF)
            dma(m2[:], m2_d[:, :, :])
            ident = consts.tile([P, P], BF)
            dma(ident[:], id_d[:, :])
            # --- tail loads (SP queue only) ---
            xTq = big.tile([P, EC, NQ * P], BF)
            for c0 in range(0, EC, 2):
                dma_sp(
                    xTq[:, c0 : c0 + 2, :],
                    xTq_d.rearrange("(c p) t -> p c t", p=P)[:, c0 : c0 + 2, :],
                )
            Wp = []
            for hf in range(2):
                wph = wpool.tile(
                    [P, EC, E // 2], BF, tag="wp", bufs=2, name=f"Wp{hf}"
                )
                for c0 in range(0, EC, 2):
                    dma_sp(
                        wph[:, c0 : c0 + 2, :],
                        WpT_d.rearrange("(c p) f -> p c f", p=P)[
                            :, c0 : c0 + 2, hf * 512 : (hf + 1) * 512
                        ],
                    )
                Wp.append(wph)
            bp_bc = consts.tile([P, E], BF)
            dma_sp(bp_bc[:], bp_d[:, :])
            gamma_bc = consts.tile([P, E], BF)
            dma_sp(gamma_bc[:], gm_d[:, :])
            beta_bc = consts.tile([P, E], BF)
            dma_sp(beta_bc[:], bt_d[:, :])

            # persistent intermediates
            KT = big.tile([P, EC, T], BF)  # K^T  [f, t]
            QT = big.tile([P, EC, NQ * P], BF)  # Q^T  [f, t_own]
            Vx = big.tile([P, NB, H, HD + 1], BF)  # V ext [t, h, d|1]
            nc.vector.memset(Vx[:, :, :, HD : HD + 1], 1.0)

            inv_e = 1.0 / float(E)

            # ---- fp8 DoubleRow projection emitters ----
            def emit_k_fb(fb, t2):
                # K^T rows for feature chunk fb, tokens t2*512..(t2+1)*512
                pk = ps.tile([P, 512], F32, tag="pz", bufs=4, name="pk")
                for half in range(2):
                    ts_ = slice((t2 * 2 + half) * 256, (t2 * 2 + half) * 256 + 256)
                    for cg in range(4):
                        nc.tensor.matmul(
                            pk[:, half * 256 : half * 256 + 256],
                            Wk[:, 2 * cg : 2 * cg + 2, fb * P : (fb + 1) * P],
                            xT8[:, 2 * cg : 2 * cg + 2, ts_],
                            start=(cg == 0),
                            stop=(cg == 3),
                            perf_mode=DR,
                        )
                nc.vector.tensor_scalar(
                    out=KT[:, fb, t2 * 512 : (t2 + 1) * 512],
                    in0=pk[:],
                    scalar1=bkT[:, fb : fb + 1],
                    scalar2=None,
                    op0=mybir.AluOpType.add,
                )

            def emit_q_fb(fb, blk):
                # Q^T rows for feature chunk fb, own query block blk
                qs = slice(blk * P, (blk + 1) * P)
                pq = ps.tile([P, 512], F32, tag="pz", bufs=4, name="pq")
                for cg in range(4):
                    nc.tensor.matmul(
                        pq[:, 0:P],
                        Wq[:, 2 * cg : 2 * cg + 2, fb * P : (fb + 1) * P],
                        xTq8[:, 2 * cg : 2 * cg + 2, qs],
                        start=(cg == 0),
                        stop=(cg == 3),
                        perf_mode=DR,
                    )
                nc.vector.tensor_scalar(
                    out=QT[:, fb, qs],
                    in0=pq[:, 0:P],
                    scalar1=bqT[:, fb : fb + 1],
                    scalar2=None,
                    op0=mybir.AluOpType.add,
                )

            def emit_v_tb(tb):
                # V rows for token block tb; first two blocks get a
                # first-order fp8 residual correction (x_r@Wv + x@Wv_r)
                passes = [(xT8, Wv)]
                if tb < 2:
                    passes += [(xr8, Wv), (xT8, Wvr)]
                for f2 in range(2):
                    pv = ps.tile([P, 512], F32, tag="pz", bufs=4, name="pv")
                    for half in range(2):
                        fs = slice((f2 * 2 + half) * 256, (f2 * 2 + half) * 256 + 256)
                        for pi, (xs, ws) in enumerate(passes):
                            xcols = slice(tb * P, (tb + 1) * P)
                            for cg in range(4):
                                nc.tensor.matmul(
                                    pv[:, half * 256 : half * 256 + 256],
                                    xs[:, 2 * cg : 2 * cg + 2, xcols],
                                    ws[:, 2 * cg : 2 * cg + 2, fs],
                                    start=(pi == 0 and cg == 0),
                                    stop=(pi == len(passes) - 1 and cg == 3),
                                    perf_mode=DR,
                                )
                    nc.vector.tensor_tensor(
                        out=Vx[:, tb, f2 * 8 : (f2 + 1) * 8, 0:HD],
                        in0=pv[:, :].rearrange("p (h d) -> p h d", d=HD),
                        in1=bv_bc[:, f2 * 512 : (f2 + 1) * 512].rearrange(
                            "p (h d) -> p h d", d=HD
                        ),
                        op=mybir.AluOpType.add,
                    )

            # ---- attention score group ----
            def emit_sgroup(pr, qs, g0, gw):
                pS = ps.tile([P, 1024], F32, tag="S", bufs=2, name="pS")
                for jj in range(gw):
                    js = slice((g0 + jj) * P, (g0 + jj + 1) * P)
                    nc.tensor.matmul(
                        pS[:, jj * P : (jj + 1) * P],
                        KT[0:64, pr, js],
                        QT[0:64, pr, qs],
                        start=True,
                        stop=True,
                        tile_position=(0, 0),
                    )
                    nc.tensor.matmul(
                        pS[:, 512 + jj * P : 512 + (jj + 1) * P],
                        KT[64:128, pr, js],
                        QT[64:128, pr, qs],
                        start=True,
                        stop=True,
                        tile_position=(64, 0),
                    )
                return pS

            units = []
            flat = []
            for k_idx in range(NQ):
                L = PAD_L[k_idx]
                for pr in range(H // 2):
                    u = len(units)
                    units.append((k_idx, pr, L))
                    for g0 in range(0, L, 4):
                        flat.append((u, g0, min(4, L - g0)))

            def sgroup_for(idx):
                u, g0, gw = flat[idx]
                k_idx, pr, L = units[u]
                return emit_sgroup(pr, slice(k_idx * P, (k_idx + 1) * P), g0, gw)

            # ---- projection + LN pipeline (runs inside window qb+1),
            # split into small thunks so injected PE work never delays the
            # score pipeline by more than ~1us ----
            ln_state = {}

            def ln_tr(qb, half):
                # transpose z[q, e] -> [e, q] via PE into a bf16 view of a
                # f32 psum tile; drain fuses the residual add. half=0 covers
                # heads 0-7 (available right after pr3's division, inside
                # window qb itself), half=1 the rest.
                qs = slice(qb * P, (qb + 1) * P)
                cs = slice(half * 4, half * 4 + 4)
                pTf = ps.tile([P, 512], F32, tag="pz", bufs=4, name="pTf")
                pT = pTf[:, :].bitcast(BF)  # [P, 1024] bf16 view
                for ci in range(4):
                    c = half * 4 + ci
                    nc.tensor.transpose(
                        pT[:, ci * P : (ci + 1) * P],
                        z_tiles[qb][:, c * P : (c + 1) * P],
                        ident[:],
                    )
                if half == 0:
                    zTq = work.tile([P, EC, P], BF, tag="zt", bufs=1, name="zTq")
                    y_sb = work.tile([P, E], BF, tag="ysb", bufs=2, name="y_sb")
                    ln_state[qb] = [zTq, y_sb]
                zTq = ln_state[qb][0]
                nc.vector.tensor_tensor(
                    out=zTq[:, cs, :],
                    in0=pT[:, 0:512].rearrange("p (c q) -> p c q", q=P),
                    in1=xTq[:, cs, qs],
                    op=mybir.AluOpType.add,
                )

            def ln_proj(qb, fs):
                zTq, y_sb = ln_state[qb][:2]
                py = ps.tile([P, 512], F32, tag="pz", bufs=4, name="py")
                for c in range(EC):
                    nc.tensor.matmul(
                        py[:],
                        zTq[:, c, :],
                        Wp[fs][:, c, :],
                        start=(c == 0),
                        stop=(c == EC - 1),
                    )
                nc.vector.tensor_tensor(
                    out=y_sb[:, fs * 512 : (fs + 1) * 512],
                    in0=py[:],
                    in1=bp_bc[:, fs * 512 : (fs + 1) * 512],
                    op=mybir.AluOpType.add,
                )

            def ln_stats(qb):
                y_sb = ln_state[qb][1]
                sm = work.tile([P, 1], F32, tag="stat", bufs=16, name="sm")
                nc.vector.reduce_sum(sm[:], y_sb[:], axis=mybir.AxisListType.X)
                negmean = work.tile([P, 1], F32, tag="stat", bufs=16, name="nm")
                nc.vector.tensor_scalar_mul(negmean[:], sm[:], -inv_e)
                ysq = work.tile([P, E], BF, tag="yc", bufs=2, name="ysq")
                s2 = work.tile([P, 1], F32, tag="stat", bufs=16, name="s2")
                nc.vector.tensor_tensor(
                    out=ysq[:], in0=y_sb[:], in1=y_sb[:], op=mybir.AluOpType.mult
                )
                nc.vector.reduce_sum(s2[:], ysq[:], axis=mybir.AxisListType.X)
                nc.vector.tensor_scalar_mul(s2[:], s2[:], inv_e)
                # var = E[y^2] - mean^2 (+eps), rstd = 1/sqrt(var)
                mu2 = work.tile([P, 1], F32, tag="stat", bufs=16, name="mu2")
                nc.vector.tensor_tensor(
                    out=mu2[:], in0=negmean[:], in1=negmean[:],
                    op=mybir.AluOpType.mult,
                )
                nc.vector.tensor_scalar(
                    out=mu2[:], in0=mu2[:], scalar1=-1.0, scalar2=float(EPS),
                    op0=mybir.AluOpType.mult, op1=mybir.AluOpType.add,
                )
                var = work.tile([P, 1], F32, tag="stat", bufs=16, name="var")
                nc.vector.tensor_tensor(
                    out=var[:], in0=s2[:], in1=mu2[:], op=mybir.AluOpType.add
                )
                rstd = work.tile([P, 1], F32, tag="stat", bufs=16, name="rstd")
                nc.scalar.activation(
                    rstd[:], var[:], mybir.ActivationFunctionType.Sqrt
                )
                nc.vector.reciprocal(rstd[:], rstd[:])
                ln_state[qb] += [negmean, rstd]

            def ln_norm(qb):
                _, y_sb, negmean, rstd = ln_state.pop(qb)
                y_c = work.tile([P, E], BF, tag="yc", bufs=2, name="y_c")
                nc.vector.tensor_scalar(
                    out=y_c[:], in0=y_sb[:], scalar1=negmean[:, 0:1], scalar2=None,
                    op0=mybir.AluOpType.add,
                )
                nc.vector.tensor_tensor(
                    out=y_c[:], in0=y_c[:], in1=gamma_bc[:], op=mybir.AluOpType.mult
                )
                nc.vector.tensor_scalar(
                    out=y_c[:], in0=y_c[:], scalar1=rstd[:, 0:1], scalar2=None,
                    op0=mybir.AluOpType.mult,
                )
                y_f = work.tile([P, E], F32, tag="yf", bufs=1, name="y_f")
                nc.vector.tensor_tensor(
                    out=y_f[:], in0=y_c[:], in1=beta_bc[:], op=mybir.AluOpType.add
                )
                nc.sync.dma_start(y_d[qb, :, :], y_f[:])

            # ---- the fused attention loop ----
            # deferred work queue: each thunk is <=~1.5us of PE work; one is
            # drained per score group so injected work never starves the exp
            # pipeline. Thunks are tagged with their origin window; all
            # thunks from before the previous window are force-drained at
            # window boundaries to keep tile-rotation WARs sound.
            pending = []

            def drain(upto_window=None, limit=1):
                n = 0
                while pending and (
                    (upto_window is not None and pending[0][0] <= upto_window)
                    or (upto_window is None and n < limit)
                ):
                    pending.pop(0)[1]()
                    n += 1

            z_tiles = []
            z_cur = None
            pz_cur = None
            emit_k_fb(0, 0)
            emit_q_fb(0, 0)
            prev_S = sgroup_for(0)
            for i, (u, g0, gw) in enumerate(flat):
                k_idx, pr, L = units[u]
                qs = slice(k_idx * P, (k_idx + 1) * P)
                h_e, h_o = 2 * pr, 2 * pr + 1
                if pr == 0 and g0 == 0:
                    drain(upto_window=k_idx - 1)
                    z_cur = work.tile([P, E], BF, tag="zsb", bufs=2, name="z_sb")
                    z_tiles.append(z_cur)
                if g0 == 0:
                    pz_cur = (
                        ps.tile([P, 512], F32, tag="pz", bufs=4, name="pE"),
                        ps.tile([P, 512], F32, tag="pz", bufs=4, name="pO"),
                    )
                pE, pO = pz_cur
                pS = prev_S
                w = gw * P
                eS = work.tile([P, 1024], BF, tag="eS", bufs=3, name="eS")
                nc.scalar.activation(
                    eS[:, :].rearrange("p (u q) -> p u q", u=2)[:, :, 0:w],
                    pS[:, :].rearrange("p (u q) -> p u q", u=2)[:, :, 0:w],
                    mybir.ActivationFunctionType.Exp,
                    scale=SCALE,
                )
                # enqueue deferred projection/LN work for later windows
                if g0 == 0:
                    if k_idx == 0 and pr + 1 < EC:
                        # K/Q chunks consumed within this window: emit
                        # directly, not via the queue
                        emit_k_fb(pr + 1, 0)
                        emit_q_fb(pr + 1, 0)
                        if pr == 0:
                            emit_v_tb(0)
                            emit_v_tb(1)
                    if pr == 1 and 2 * k_idx + 2 < NB:
                        tb = 2 * k_idx + 2
                        pending.append((k_idx, lambda tb=tb: emit_v_tb(tb)))
                    if pr == 3 and 2 * k_idx + 3 < NB:
                        tb = 2 * k_idx + 3
                        pending.append((k_idx, lambda tb=tb: emit_v_tb(tb)))
                    if pr == 2 and k_idx < NQ - 1:
                        # Q rows for the next window's query block
                        blk = k_idx + 1

                        def _q(blk=blk):
                            for fb in range(EC):
                                emit_q_fb(fb, blk)

                        pending.append((k_idx, _q))
                    if pr in (2, 3, 4, 5) and k_idx in (0, 2, 4):
                        # K token chunks ahead of the window that needs them,
                        # spread across units to avoid a boundary burst
                        t2 = k_idx // 2 + 1
                        for fb in (2 * (pr - 2), 2 * (pr - 2) + 1):
                            pending.append(
                                (k_idx + 1, lambda fb=fb, t2=t2: emit_k_fb(fb, t2))
                            )
                    if pr == 4:
                        # first half of this window's z is complete
                        pending.append((k_idx, lambda qb=k_idx: ln_tr(qb, 0)))
                    if k_idx >= 1:
                        qb = k_idx - 1
                        if pr == 0:
                            pending.append((k_idx, lambda qb=qb: ln_tr(qb, 1)))
                            pending.append((k_idx, lambda qb=qb: ln_proj(qb, 0)))
                        elif pr == 1:
                            pending.append((k_idx, lambda qb=qb: ln_proj(qb, 1)))
                        elif pr == 2:
                            pending.append((k_idx, lambda qb=qb: ln_stats(qb)))
                        elif pr == 3:
                            pending.append((k_idx, lambda qb=qb: ln_norm(qb)))
                if i + 1 < len(flat):
                    # next score group issues on PE while ACT runs this exp;
                    # at window transitions, first force-drain everything the
                    # next window's score groups may read
                    nk = units[flat[i + 1][0]][0]
                    if nk != k_idx:
                        drain(upto_window=nk - 1)
                    prev_S = sgroup_for(i + 1)
                drain(limit=1)
                for jj in range(gw):
                    j = g0 + jj
                    if j >= L - 2:
                        m = m1 if j == L - 2 else m2
                        nc.gpsimd.tensor_tensor(
                            out=eS[:, :].rearrange("p (u q) -> p u q", u=2)[
                                :, :, jj * P : (jj + 1) * P
                            ],
                            in0=eS[:, :].rearrange("p (u q) -> p u q", u=2)[
                                :, :, jj * P : (jj + 1) * P
                            ],
                            in1=m[:, k_idx : k_idx + 1, :].to_broadcast((P, 2, P)),
                            op=mybir.AluOpType.mult,
                        )
                    for h, uu, zP in ((h_e, 0, pE), (h_o, 1, pO)):
                        nc.tensor.matmul(
                            zP[:, 0 : HD + 1],
                            eS[:, :].rearrange("p (u q) -> p u q", u=2)[
                                :, uu, jj * P : (jj + 1) * P
                            ],
                            Vx[:, j, h, :],
                            start=(j == 0),
                            stop=(j == L - 1),
                        )
                if g0 + gw == L:
                    # softmax division: denominator is per-partition (per-q)
                    for h, zP in ((h_e, pE), (h_o, pO)):
                        rs = work.tile([P, 1], F32, tag="rs", bufs=4, name="rs")
                        nc.vector.reciprocal(rs[:], zP[:, HD : HD + 1])
                        nc.vector.tensor_scalar(
                            out=z_cur[:, h * HD : (h + 1) * HD],
                            in0=zP[:, 0:HD],
                            scalar1=rs[:, 0:1],
                            scalar2=None,
                            op0=mybir.AluOpType.mult,
                        )
            # tail: drain leftovers and finish LN for the last query block
            drain(upto_window=NQ)
            ln_tr(NQ - 1, 1)
            ln_proj(NQ - 1, 0)
            ln_proj(NQ - 1, 1)
            ln_stats(NQ - 1)
            ln_norm(NQ - 1)

    _nc_cache["nc"] = nc
    return nc


def _make_masks(blocks):
    m1 = np.zeros((NQ, P, P), np.float32)
    m2 = np.zeros((NQ, P, P), np.float32)
    tril_t = (np.arange(P)[:, None] <= np.arange(P)[None, :]).astype(np.float32)
    for k in range(NQ):
        l_true = blocks[k] + 1
        L = PAD_L[k]
        if l_true == L:
            m1[k] = 1.0
            m2[k] = tril_t
        else:
            assert l_true == L - 1
            m1[k] = tril_t
            m2[k] = 0.0
    # device layout [P(k-local), NQ, P(q-local)]
    return (
        np.ascontiguousarray(m1.transpose(1, 0, 2)).astype(NPBF),
        np.ascontiguousarray(m2.transpose(1, 0, 2)).astype(NPBF),
    )


def kernel(x, Wq, bq, Wk, bk, Wv, bv, Wp, bp, gamma, beta):
    x = np.asarray(x, np.float32)
    nc = _build_nc()

    Wq8 = np.ascontiguousarray(np.asarray(Wq, np.float32).T).astype(NPF8)
    Wk8 = np.ascontiguousarray(np.asarray(Wk, np.float32).T).astype(NPF8)
    Wv8 = np.ascontiguousarray(np.asarray(Wv, np.float32).T).astype(NPF8)
    Wvr8 = (
        np.ascontiguousarray(np.asarray(Wv, np.float32).T)
        - Wv8.astype(np.float32)
    ).astype(NPF8)
    WpT = np.ascontiguousarray(np.asarray(Wp, np.float32).T).astype(NPBF)
    bqT = np.ascontiguousarray(np.asarray(bq, np.float32).reshape(EC, P).T)
    bkT = np.ascontiguousarray(np.asarray(bk, np.float32).reshape(EC, P).T)
    bv_bc = np.ascontiguousarray(
        np.broadcast_to(np.asarray(bv, np.float32), (P, E))
    ).astype(NPBF)
    bp_bc = np.ascontiguousarray(
        np.broadcast_to(np.asarray(bp, np.float32), (P, E))
    ).astype(NPBF)
    gamma_bc = np.ascontiguousarray(
        np.broadcast_to(np.asarray(gamma, np.float32), (P, E))
    ).astype(NPBF)
    beta_bc = np.ascontiguousarray(
        np.broadcast_to(np.asarray(beta, np.float32), (P, E))
    ).astype(NPBF)
    ident = np.eye(P, dtype=np.float32).astype(NPBF)
    masks = {0: _make_masks(BLOCKS_A), 1: _make_masks(BLOCKS_B)}

    in_maps = []
    for core in range(8):
        b, h = core // 2, core % 2
        blocks = BLOCKS_A if h == 0 else BLOCKS_B
        own = np.concatenate([np.arange(blk * P, (blk + 1) * P) for blk in blocks])
        xbT = np.ascontiguousarray(x[b].T)
        xT8_np = xbT.astype(NPF8)
        xr8_np = (
            xbT[:, 0 : 2 * P] - xT8_np[:, 0 : 2 * P].astype(np.float32)
        ).astype(NPF8)
        m1c, m2c = masks[h]
        in_maps.append(
            {
                "xT8": xT8_np,
                "xr8": xr8_np,
                "Wvr8": Wvr8,
                "xTq8": np.ascontiguousarray(xbT[:, own]).astype(NPF8),
                "xTq": np.ascontiguousarray(xbT[:, own]).astype(NPBF),
                "Wq8": Wq8,
                "Wk8": Wk8,
                "Wv8": Wv8,
                "WpT": WpT,
                "bqT": bqT,
                "bkT": bkT,
                "bv_bc": bv_bc,
                "bp_bc": bp_bc,
                "gamma_bc": gamma_bc,
                "beta_bc": beta_bc,
                "m1": m1c,
                "m2": m2c,
                "ident": ident,
            }
        )

    res = run_bass_kernel_spmd(nc, in_maps, core_ids=list(range(8)))

    out = np.empty((B, T, E), np.float32)
    for core in range(8):
        b, h = core // 2, core % 2
        blocks = BLOCKS_A if h == 0 else BLOCKS_B
        y = res.results[core]["y"]  # (NQ, P, E)
        for k, blk in enumerate(blocks):
            out[b, blk * P : (blk + 1) * P, :] = y[k]
    return out
